# revision 42
# baseline (speedup 1.0000x reference)
"""Trainium2 Bass kernel for nn_DenseContrastive (dense contrastive loss).

Math (per the fused reference):
    A = anchors (N, c), E = ema features (N, c), N = 12800, c = 64
    pos_i   = (A_i . E_i) / TEMP
    l_ij    = (A_i . E_j) / TEMP
    den_i   = sum_j exp(l_ij - s_i)            (s_i = per-row shift)
    r_i     = e1_i / (den_i + EPS + e1_i),  e1_i = exp(pos_i - s_i)
    loss_i  = -log(r_i + EPS);   out = mean_i loss_i

Shift choice (the structural win): s_i = pos_i + MARGIN, known on the
host -- no row-max pass.  The denominator always contains the diagonal
term exp(-MARGIN), so r <= ~1/2; if any exp overflows (row max > ~88
logits above s) then den saturates and loss = -log(EPS), which is
exactly what the reference computes for such a row (its softmax ratio
underflows below EPS).  Verified to rel err ~5e-7 vs the fp32 reference.

PE tiling (the throughput win): K = 64 contraction channels only fills
half the 128-row PE array, and a 64-partition moving stream runs at half
rate.  A and E are duplicated into both partition halves and row tiles
are processed in PAIRS: tile_position (0,0) computes row-tile t0 on PE
rows 0-63 while (64,0) computes t1 on rows 64-127, concurrently
(measured ~3x matmul throughput vs unpaired).

exp work is split across ACT and DVE per row tile (25 x 512 blocks,
15/10 and 18/7 on alternating pairs to balance engine busy time):
  ACT: table exp of scale*PSUM + bias_row (per-partition AP);
      accum_out gives the row sums for free.
  DVE: bf16 Schraudolph -- i16 = rne(max(PSUM + Brow, 0)), whose bit
      pattern read as bf16 is ~exp(10x) (ALPHA = 10*log2e*2^7 is folded
      into A; Brow = B16 - ALPHA*(pos+MARGIN), a per-partition scalar
      AP, folds the exponent bias and the row shift; the high side
      saturates the int16 convert to 0x7FFF = bf16 NaN, which a final
      min(den, 3e38) scrubs -- DVE min takes the non-NaN operand, and
      such rows are saturated anyway).  Row sums run as a packed-bf16
      tensor_tensor ADD tree (2 elem/cycle) plus a short 1x reduce.
e1 is computed on the host from the same shift, so numerator and
denominator shifts cancel exactly.

Sharding: N rows split across 8 cores (1600 each); E' (128 x 12800 bf16,
duplicated halves) replicated per core.  Core returns sum_i log(r_i+EPS);
host combines: loss = -sum/N.
"""

import sys

for _p in ("/opt/trn_rl_repo",):
    if _p not in sys.path:
        sys.path.insert(0, _p)

import numpy as np

import concourse.bass as bass
import concourse.bacc as bacc
import concourse.tile as tile
from concourse import mybir

TEMP = 0.1
EPS = 1e-8
B_, C, H, W = 2, 64, 80, 80
N = B_ * H * W           # 12800 anchors
NCORES = 8
R = N // NCORES          # 1600 rows per core
BLK = 512                # logit columns per PSUM bank
NBLK = N // BLK          # 25
MARGIN = 0.5             # raw-dot units; e1 ~ e^-5

LOG2E = 1.4426950408889634
# bf16-domain Schraudolph: exp(10*x) ~ bitcast16 of round(ALPHA*x + Brow)
ALPHA = np.float32(10.0 * LOG2E * (1 << 7))           # 1846.6497
SCHRAUD_C = 486408.0 / 65536.0                        # ~7.42
BCONST = np.float32(127.0 * (1 << 7) - SCHRAUD_C)     # ~16248.58
ACT_SCALE = np.float32(10.0 / float(ALPHA))
DEN_CAP = 3.0e38

F32 = mybir.dt.float32
I16 = mybir.dt.int16
BF16 = mybir.dt.bfloat16

# 1600 rows -> 6 pairs of 128-row tiles + one 64-row tile
ROW_TILES = [(i * 128, 128) for i in range(12)] + [(1536, 64)]
NRT = len(ROW_TILES)
NSLOT = 11               # up to 6 ACT accum slots + 5 reduce slots

# Alternating per-pair block plans balance ACT vs DVE at ~16.6/8.4:
# PLAN_A = 15 ACT / 10 DVE, PLAN_B = 18 ACT / 7 DVE.
PLAN_A = [
    ((0, 1, 2), (3, 4)),
    ((5, 6, 7), (8, 9)),
    ((10, 11, 12), (13, 14)),
    ((15, 16, 17), (18, 19)),
    ((20, 21, 22), (23, 24)),
]
PLAN_B = [
    ((0, 1, 2), (3, 4)),
    ((5, 6, 7), (8, 9)),
    ((10, 11, 12), (13, 14)),
    ((15, 16, 17), (18,)),
    ((19, 20, 21), ()),
    ((22, 23, 24), ()),
]


def _build() -> bass.Bass:
    nc = bacc.Bacc("TRN2", target_bir_lowering=False)
    ae_tb = nc.declare_dram_parameter("ae_tb", [128, R], BF16, isOutput=False)
    e_tb = nc.declare_dram_parameter("e_tb", [128, N], BF16, isOutput=False)
    bias_in = nc.declare_dram_parameter("bias_in", [128, NRT], F32, isOutput=False)
    e1_in = nc.declare_dram_parameter("e1_in", [128, NRT], F32, isOutput=False)
    out = nc.declare_dram_parameter("out", [1, 1], F32, isOutput=True)

    exp_f = mybir.ActivationFunctionType.Exp
    ln_f = mybir.ActivationFunctionType.Ln
    op_add = mybir.AluOpType.add
    op_max = mybir.AluOpType.max
    op_min = mybir.AluOpType.min
    op_mult = mybir.AluOpType.mult

    with tile.TileContext(nc) as tc:
        with (
            tc.tile_pool(name="big", bufs=1) as big,
            tc.tile_pool(name="small", bufs=1) as small,
            tc.tile_pool(name="trash", bufs=2) as trash,
            tc.tile_pool(name="intb", bufs=4) as intb,
            tc.tile_pool(name="tadd", bufs=2) as tadd,
            tc.tile_pool(name="psA", bufs=2, space="PSUM") as psA,
            tc.tile_pool(name="psD", bufs=1, space="PSUM") as psD,
        ):
            # ---- resident SBUF data -------------------------------------
            et_b = big.tile([128, N], BF16)     # E' dup halves (3.3 MB)
            aet_b = big.tile([128, R], BF16)    # A' shard, dup halves
            bias_sb = small.tile([128, NRT], F32)
            e1_sb = small.tile([128, NRT], F32)
            nc.sync.dma_start(out=aet_b[:], in_=ae_tb[:])
            nc.sync.dma_start(out=bias_sb[:], in_=bias_in[:])
            nc.sync.dma_start(out=e1_sb[:], in_=e1_in[:])
            for k in range(8):
                s = slice(k * (N // 8), (k + 1) * (N // 8))
                nc.sync.dma_start(out=et_b[:, s], in_=e_tb[:, s])

            ones_p = small.tile([128, 1], F32)
            nc.vector.memset(ones_p, 1.0)

            # Schraudolph per-row exponent bias: Brow = (ALPHA/10)*bias + B
            b_rows = small.tile([128, NRT], F32)
            nc.vector.tensor_scalar(
                out=b_rows[:], in0=bias_sb[:],
                scalar1=float(ALPHA) / 10.0, scalar2=float(BCONST),
                op0=op_mult, op1=op_add,
            )
            dsums = small.tile([128, NRT * NSLOT], F32)
            nc.vector.memset(dsums[:], 0.0)

            def emit_tile_pair(t0, t1, plan):
                """Row tiles t0 (PE rows 0-63) and t1 (rows 64-127), paired.
                t1 may be None (odd tail tile, unpaired)."""
                tiles = [(t0, 0)] + ([(t1, 64)] if t1 is not None else [])
                for g, (blocks, dblocks) in enumerate(plan):
                    nb = len(blocks)
                    nd = len(dblocks)
                    its = []
                    # Per tile: psA mms -> ACT, then psD unit mms -> one wide
                    # p2.  psD has a single 2-bank buffer; the p2 of tile k
                    # is covered in PE program order by tile k+1's matmuls.
                    for k, (t, base) in enumerate(tiles):
                        r0, p = ROW_TILES[t]
                        kw = {} if t1 is None else {"tile_position": (base, 0)}
                        pst = psA.tile([128, 3 * BLK], F32, tag="psa",
                                       name=f"psa_{t}_{g}")
                        for j, b in enumerate(blocks):
                            nc.tensor.matmul(
                                out=pst[:p, j * BLK : (j + 1) * BLK],
                                lhsT=aet_b[base : base + 64, r0 : r0 + p],
                                rhs=et_b[base : base + 64,
                                         b * BLK : (b + 1) * BLK],
                                start=True, stop=True, **kw,
                            )
                        scr = trash.tile([128, 3 * BLK], BF16, tag="scr")
                        nc.scalar.activation(
                            out=scr[:p, 0 : nb * BLK],
                            in_=pst[:p, 0 : nb * BLK],
                            func=exp_f,
                            scale=float(ACT_SCALE),
                            bias=bias_sb[:p, t : t + 1],
                            accum_out=dsums[:p, t * NSLOT + g : t * NSLOT + g + 1],
                        )
                        if not dblocks:
                            continue
                        it = intb.tile([128, 2 * BLK], I16, tag="intb",
                                       name=f"intb_{t}_{g}")
                        its.append(it)
                        psd = psD.tile([128, 2 * BLK], F32, tag="psd")
                        for h, b in enumerate(dblocks):
                            nc.tensor.matmul(
                                out=psd[:p, h * BLK : (h + 1) * BLK],
                                lhsT=aet_b[base : base + 64, r0 : r0 + p],
                                rhs=et_b[base : base + 64,
                                         b * BLK : (b + 1) * BLK],
                                start=True, stop=True, **kw,
                            )
                        nc.vector.tensor_scalar(
                            out=it[:p, 0 : nd * BLK],
                            in0=psd[:p, 0 : nd * BLK],
                            scalar1=b_rows[:p, t : t + 1],
                            scalar2=0.0,
                            op0=op_add, op1=op_max,
                        )
                    if not dblocks:
                        continue
                    for k, (t, base) in enumerate(tiles):
                        r0, p = ROW_TILES[t]
                        slot = t * NSLOT + 6 + g
                        # packed-bf16 tree-add (2 elem/cycle) + short reduce
                        w = nd * BLK
                        v = its[k][:p, 0:w].bitcast(BF16)
                        h1 = tadd.tile([128, BLK], BF16, tag="h1")
                        nc.vector.tensor_add(
                            h1[:p, 0 : w // 2], v[:, 0 : w // 2],
                            v[:, w // 2 :],
                        )
                        h2 = tadd.tile([128, BLK // 2], BF16, tag="h2")
                        nc.vector.tensor_add(
                            h2[:p, 0 : w // 4], h1[:p, 0 : w // 4],
                            h1[:p, w // 4 : w // 2],
                        )
                        nc.vector.tensor_reduce(
                            out=dsums[:p, slot : slot + 1],
                            in_=h2[:p, 0 : w // 4],
                            axis=mybir.AxisListType.X,
                            op=op_add,
                        )

            # Alternating 15A/10D and 18A/7D pairs balances ACT vs DVE
            # busy time (measured best at ~155 us span)
            for pair in range(6):
                emit_tile_pair(2 * pair, 2 * pair + 1,
                               PLAN_A if pair % 2 == 0 else PLAN_B)
            emit_tile_pair(12, None, PLAN_B)

            # ---- tail: per-row loss and core-level sum ------------------
            den = small.tile([128, NRT], F32)
            nc.vector.tensor_reduce(
                out=den[:],
                in_=dsums[:].rearrange("p (t s) -> p t s", s=NSLOT),
                axis=mybir.AxisListType.X,
                op=op_add,
            )
            # scrub int32-saturation NaNs (saturated rows) to a huge finite
            den2 = small.tile([128, NRT], F32)
            nc.vector.tensor_scalar_min(out=den2[:], in0=den[:], scalar1=DEN_CAP)
            den_eps = small.tile([128, NRT], F32)
            nc.vector.scalar_tensor_tensor(
                out=den_eps[:], in0=den2[:], scalar=EPS, in1=e1_sb[:],
                op0=op_add, op1=op_add,
            )
            recip = small.tile([128, NRT], F32)
            nc.vector.reciprocal(out=recip[:], in_=den_eps[:])
            ratio = small.tile([128, NRT], F32)
            nc.vector.tensor_mul(ratio[:], e1_sb[:], recip[:])
            rateps = small.tile([128, NRT], F32)
            nc.vector.tensor_scalar_add(out=rateps[:], in0=ratio[:], scalar1=EPS)
            nc.vector.memset(rateps[64:128, NRT - 1 : NRT], 1.0)
            logv = small.tile([128, NRT], F32)
            lsum = small.tile([128, 1], F32)
            nc.scalar.activation(
                out=logv[:], in_=rateps[:], func=ln_f, accum_out=lsum[:]
            )
            tot_ps = psD.tile([128, 2 * BLK], F32, tag="psd")
            nc.tensor.matmul(
                out=tot_ps[0:1, 0:1], lhsT=lsum[:], rhs=ones_p[:],
                start=True, stop=True,
            )
            tot_sb = small.tile([1, 1], F32)
            nc.vector.tensor_copy(out=tot_sb[:], in_=tot_ps[0:1, 0:1])
            nc.sync.dma_start(out=out[:], in_=tot_sb[:])

    if not nc.is_finalized():
        nc.finalize()
    return nc


_NC_CACHE: list = []


def _get_nc() -> bass.Bass:
    if not _NC_CACHE:
        _NC_CACHE.append(_build())
    return _NC_CACHE[0]


_RUNNER_CACHE: list = []


def _get_runner():
    """Build the sharded PJRT executable once and reuse it across calls."""
    if _RUNNER_CACHE:
        return _RUNNER_CACHE[0]

    import jax
    import numpy as _np
    from jax.sharding import Mesh, PartitionSpec
    from jax.experimental.shard_map import shard_map
    from concourse import mybir as _mybir
    from concourse.bass2jax import (
        _bass_exec_p,
        install_neuronx_cc_hook,
        partition_id_tensor,
    )

    nc = _get_nc()
    install_neuronx_cc_hook()
    partition_name = nc.partition_id_tensor.name if nc.partition_id_tensor else None

    in_names, out_names, out_avals, zero_outs = [], [], [], []
    for alloc in nc.m.functions[0].allocations:
        if not isinstance(alloc, _mybir.MemoryLocationSet):
            continue
        name = alloc.memorylocations[0].name
        if alloc.kind == "ExternalInput":
            if name != partition_name:
                in_names.append(name)
        elif alloc.kind == "ExternalOutput":
            shape = tuple(alloc.tensor_shape)
            dtype = _mybir.dt.np(alloc.dtype)
            out_names.append(name)
            out_avals.append(jax.core.ShapedArray(shape, dtype))
            zero_outs.append(_np.zeros(shape, dtype))
    n_params = len(in_names)
    n_outs = len(out_avals)
    all_in_names = list(in_names) + list(out_names)
    if partition_name is not None:
        all_in_names.append(partition_name)

    def _body(*args):
        operands = list(args)
        if partition_name is not None:
            operands.append(partition_id_tensor())
        outs = _bass_exec_p.bind(
            *operands,
            out_avals=tuple(out_avals),
            in_names=tuple(all_in_names),
            out_names=tuple(out_names),
            lowering_input_output_aliases=(),
            sim_require_finite=False,
            sim_require_nnan=False,
            nc=nc,
        )
        return tuple(outs)

    devices = jax.devices()[:NCORES]
    mesh = Mesh(_np.asarray(devices), ("core",))
    spec_of = {
        "ae_tb": PartitionSpec("core"),
        "bias_in": PartitionSpec("core"),
        "e1_in": PartitionSpec("core"),
        "e_tb": PartitionSpec(),
    }
    in_specs = tuple(spec_of[nm] for nm in in_names) + (
        PartitionSpec("core"),
    ) * n_outs
    out_specs = (PartitionSpec("core"),) * n_outs
    donate = tuple(range(n_params, n_params + n_outs))
    sharded = jax.jit(
        shard_map(
            _body, mesh=mesh, in_specs=in_specs, out_specs=out_specs, check_rep=False
        ),
        donate_argnums=donate,
        keep_unused=True,
    )

    state = (sharded, in_names, out_names, out_avals, zero_outs)
    _RUNNER_CACHE.append(state)
    return state


def _to_bf16(x: np.ndarray):
    import ml_dtypes

    return x.astype(ml_dtypes.bfloat16)


def _prep_feeds(proj_main, proj_ema):
    """Full inputs -> (ae stacked, bias stacked, e1 stacked, e2 bf16)."""
    import ml_dtypes

    pm = np.ascontiguousarray(np.asarray(proj_main, dtype=np.float32))
    pe = np.ascontiguousarray(np.asarray(proj_ema, dtype=np.float32))
    at_full = np.ascontiguousarray(pm.transpose(1, 0, 2, 3).reshape(C, N))
    et_full = np.ascontiguousarray(pe.transpose(1, 0, 2, 3).reshape(C, N))

    pos = (at_full * et_full).sum(axis=0, dtype=np.float32)  # (N,) raw dots

    # per-row shift m' = pos + MARGIN (fp32); ACT bias = -10*m'
    bias_act = (-10.0 * (pos + np.float32(MARGIN))).astype(np.float32)
    e1 = np.exp(10.0 * pos.astype(np.float64)
                + bias_act.astype(np.float64)).astype(np.float32)

    a_scaled = _to_bf16(at_full * np.float32(ALPHA))     # (64, N) bf16
    e_b16 = _to_bf16(et_full)                            # (64, N) bf16
    ae_full = np.empty((128, N), dtype=ml_dtypes.bfloat16)
    ae_full[0:64] = a_scaled
    ae_full[64:128] = a_scaled
    e2 = np.empty((128, N), dtype=ml_dtypes.bfloat16)
    e2[0:64] = e_b16
    e2[64:128] = e_b16

    def layout_rows(v, pad=0.0):
        vp = np.full(NCORES * NRT * 128, pad, dtype=np.float32)
        for core in range(NCORES):
            vp[core * NRT * 128 : core * NRT * 128 + R] = v[
                core * R : (core + 1) * R
            ]
        return np.ascontiguousarray(
            vp.reshape(NCORES, NRT, 128).transpose(0, 2, 1).reshape(
                NCORES * 128, NRT
            )
        )

    bias_in = layout_rows(bias_act)
    e1_in = layout_rows(e1, pad=0.0)
    ae_sh = np.ascontiguousarray(
        np.asarray(ae_full).reshape(128, NCORES, R).transpose(1, 0, 2).reshape(
            NCORES * 128, R
        )
    )
    return ae_sh, bias_in, e1_in, np.ascontiguousarray(e2)


def _trace_in_maps(np_inputs):
    """Per-core input dicts for run_bass_kernel_spmd (trace harness)."""
    ae_sh, bias_in, e1_in, e2 = _prep_feeds(
        np_inputs["proj_main"], np_inputs["proj_ema"]
    )
    maps = []
    for core in range(NCORES):
        maps.append(
            {
                "ae_tb": np.ascontiguousarray(
                    ae_sh[core * 128 : (core + 1) * 128]
                ),
                "bias_in": np.ascontiguousarray(
                    bias_in[core * 128 : (core + 1) * 128]
                ),
                "e1_in": np.ascontiguousarray(
                    e1_in[core * 128 : (core + 1) * 128]
                ),
                "e_tb": e2,
            }
        )
    return maps


def kernel(proj_main, proj_ema, label_main, label_ema, patch_num):
    # labels / patch_num never influence the loss; only the projections do.
    ae_sh, bias_in, e1_in, e2 = _prep_feeds(proj_main, proj_ema)

    sharded, in_names, out_names, out_avals, zero_outs = _get_runner()
    feed = {"ae_tb": ae_sh, "bias_in": bias_in, "e1_in": e1_in, "e_tb": e2}
    args = [feed[nm] for nm in in_names]
    args += [
        np.zeros((NCORES * z.shape[0], *z.shape[1:]), z.dtype) for z in zero_outs
    ]
    out_arrs = sharded(*args)
    outs = np.asarray(out_arrs[out_names.index("out")]).reshape(NCORES)
    return np.float32(-float(outs.sum()) / N)


if __name__ == "__main__":
    _build()
    print("build OK")


# revision 45
# speedup vs baseline: 1.2650x; 1.2650x over previous
"""Trainium2 Bass kernel for nn_DenseContrastive (dense contrastive loss).

Math (per the fused reference):
    A = anchors (N, c), E = ema features (N, c), N = 12800, c = 64
    pos_i   = (A_i . E_i) / TEMP
    l_ij    = (A_i . E_j) / TEMP
    den_i   = sum_j exp(l_ij - s_i)            (s_i = per-row shift)
    r_i     = e1_i / (den_i + EPS + e1_i),  e1_i = exp(pos_i - s_i)
    loss_i  = -log(r_i + EPS);   out = mean_i loss_i

Shift choice (the structural win): s_i = pos_i + MARGIN, known on the
host -- no row-max pass.  The denominator always contains the diagonal
term exp(-MARGIN), so r <= ~1/2; if any exp overflows (row max > ~88
logits above s) then den saturates and loss = -log(EPS), which is
exactly what the reference computes for such a row (its softmax ratio
underflows below EPS).  Verified to rel err ~5e-7 vs the fp32 reference.

PE tiling (the throughput win): K = 64 contraction channels only fills
half the 128-row PE array, and a 64-partition moving stream runs at half
rate.  A and E are duplicated into both partition halves and row tiles
are processed in PAIRS: tile_position (0,0) computes row-tile t0 on PE
rows 0-63 while (64,0) computes t1 on rows 64-127, concurrently
(measured ~3x matmul throughput vs unpaired).

exp work is split across ACT and DVE per row tile (25 x 512 blocks,
15/10 and 18/7 on alternating pairs to balance engine busy time):
  ACT: table exp of scale*PSUM + bias_row (per-partition AP);
      accum_out gives the row sums for free.
  DVE: bf16 Schraudolph -- i16 = rne(max(PSUM + Brow, 0)), whose bit
      pattern read as bf16 is ~exp(10x) (ALPHA = 10*log2e*2^7 is folded
      into A; Brow = B16 - ALPHA*(pos+MARGIN), a per-partition scalar
      AP, folds the exponent bias and the row shift; the high side
      saturates the int16 convert to 0x7FFF = bf16 NaN, which a final
      min(den, 3e38) scrubs -- DVE min takes the non-NaN operand, and
      such rows are saturated anyway).  Row sums run as a packed-bf16
      tensor_tensor ADD tree (2 elem/cycle) plus a short 1x reduce.
e1 is computed on the host from the same shift, so numerator and
denominator shifts cancel exactly.

Sharding: N rows split across 8 cores (1600 each); E' (128 x 12800 bf16,
duplicated halves) replicated per core.  Core returns sum_i log(r_i+EPS);
host combines: loss = -sum/N.
"""

import sys

for _p in ("/opt/trn_rl_repo",):
    if _p not in sys.path:
        sys.path.insert(0, _p)

import numpy as np

import concourse.bass as bass
import concourse.bacc as bacc
import concourse.tile as tile
from concourse import mybir

TEMP = 0.1
EPS = 1e-8
B_, C, H, W = 2, 64, 80, 80
N = B_ * H * W           # 12800 anchors
NCORES = 8
R = N // NCORES          # 1600 rows per core
BLK = 512                # logit columns per PSUM bank
NBLK = N // BLK          # 25
MARGIN = 0.5             # raw-dot units; e1 ~ e^-5

LOG2E = 1.4426950408889634
# bf16-domain Schraudolph: exp(10*x) ~ bitcast16 of round(ALPHA*x + Brow)
ALPHA = np.float32(10.0 * LOG2E * (1 << 7))           # 1846.6497
SCHRAUD_C = 486408.0 / 65536.0                        # ~7.42
BCONST = np.float32(127.0 * (1 << 7) - SCHRAUD_C)     # ~16248.58
ACT_SCALE = np.float32(10.0 / float(ALPHA))
DEN_CAP = 3.0e38

F32 = mybir.dt.float32
I16 = mybir.dt.int16
BF16 = mybir.dt.bfloat16

# 1600 rows -> 6 pairs of 128-row tiles + one 64-row tile
ROW_TILES = [(i * 128, 128) for i in range(12)] + [(1536, 64)]
NRT = len(ROW_TILES)
NSLOT = 11               # up to 6 ACT accum slots + 5 reduce slots

# Uniform 17 ACT / 8 DVE blocks per tile (the measured busy-time
# optimum), with D work spread across groups to keep DVE load smooth.
PLAN_U = [
    ((0, 1, 2), (3, 4)),
    ((5, 6, 7), (8, 9)),
    ((10, 11, 12), (13, 14)),
    ((15, 16, 17), (18,)),
    ((19, 20, 21), (22,)),
    ((23, 24), ()),
]


def _build() -> bass.Bass:
    nc = bacc.Bacc("TRN2", target_bir_lowering=False)
    ae_tb = nc.declare_dram_parameter("ae_tb", [128, R], BF16, isOutput=False)
    e_tb = nc.declare_dram_parameter("e_tb", [128, N], BF16, isOutput=False)
    bias_in = nc.declare_dram_parameter("bias_in", [128, NRT], F32, isOutput=False)
    e1_in = nc.declare_dram_parameter("e1_in", [128, NRT], F32, isOutput=False)
    out = nc.declare_dram_parameter("out", [1, 1], F32, isOutput=True)

    exp_f = mybir.ActivationFunctionType.Exp
    ln_f = mybir.ActivationFunctionType.Ln
    op_add = mybir.AluOpType.add
    op_max = mybir.AluOpType.max
    op_min = mybir.AluOpType.min
    op_mult = mybir.AluOpType.mult

    with tile.TileContext(nc) as tc:
        with (
            tc.tile_pool(name="big", bufs=1) as big,
            tc.tile_pool(name="small", bufs=1) as small,
            tc.tile_pool(name="trash", bufs=2) as trash,
            tc.tile_pool(name="intb", bufs=4) as intb,
            tc.tile_pool(name="tadd", bufs=2) as tadd,
            tc.tile_pool(name="psA", bufs=2, space="PSUM") as psA,
            tc.tile_pool(name="psD", bufs=2, space="PSUM") as psD,
        ):
            # ---- resident SBUF data -------------------------------------
            et_b = big.tile([128, N], BF16)     # E' dup halves (3.3 MB)
            aet_b = big.tile([128, R], BF16)    # A' shard, dup halves
            bias_sb = small.tile([128, NRT], F32)
            e1_sb = small.tile([128, NRT], F32)
            nc.sync.dma_start(out=aet_b[:], in_=ae_tb[:])
            nc.sync.dma_start(out=bias_sb[:], in_=bias_in[:])
            nc.sync.dma_start(out=e1_sb[:], in_=e1_in[:])
            for k in range(8):
                s = slice(k * (N // 8), (k + 1) * (N // 8))
                nc.sync.dma_start(out=et_b[:, s], in_=e_tb[:, s])

            ones_p = small.tile([128, 1], F32)
            nc.vector.memset(ones_p, 1.0)

            # Schraudolph per-row exponent bias: Brow = (ALPHA/10)*bias + B
            b_rows = small.tile([128, NRT], F32)
            nc.vector.tensor_scalar(
                out=b_rows[:], in0=bias_sb[:],
                scalar1=float(ALPHA) / 10.0, scalar2=float(BCONST),
                op0=op_mult, op1=op_add,
            )
            dsums = small.tile([128, NRT * NSLOT], F32)
            nc.vector.memset(dsums[:], 0.0)

            def emit_tile_pair(t0, t1, plan):
                """Row tiles t0 (PE rows 0-63) and t1 (rows 64-127), paired.
                t1 may be None (odd tail tile, unpaired)."""
                tiles = [(t0, 0)] + ([(t1, 64)] if t1 is not None else [])
                for g, (blocks, dblocks) in enumerate(plan):
                    psas, its = [], []
                    for (t, base) in tiles:
                        psas.append(
                            psA.tile([128, 3 * BLK], F32, tag="psa",
                                     name=f"psa_{t}_{g}")
                        )
                    for j, b in enumerate(blocks):
                        for k, (t, base) in enumerate(tiles):
                            r0, p = ROW_TILES[t]
                            kw = {} if t1 is None else {
                                "tile_position": (base, 0)}
                            nc.tensor.matmul(
                                out=psas[k][:p, j * BLK : (j + 1) * BLK],
                                lhsT=aet_b[base : base + 64, r0 : r0 + p],
                                rhs=et_b[base : base + 64,
                                         b * BLK : (b + 1) * BLK],
                                start=True, stop=True, **kw,
                            )
                    nb = len(blocks)
                    for k, (t, base) in enumerate(tiles):
                        r0, p = ROW_TILES[t]
                        scr = trash.tile([128, 3 * BLK], BF16, tag="scr")
                        nc.scalar.activation(
                            out=scr[:p, 0 : nb * BLK],
                            in_=psas[k][:p, 0 : nb * BLK],
                            func=exp_f,
                            scale=float(ACT_SCALE),
                            bias=bias_sb[:p, t : t + 1],
                            accum_out=dsums[:p, t * NSLOT + g : t * NSLOT + g + 1],
                        )
                    if not dblocks:
                        continue
                    nd = len(dblocks)
                    for (t, base) in tiles:
                        its.append(
                            intb.tile([128, 2 * BLK], I16, tag="intb",
                                      name=f"intb_{t}_{g}")
                        )
                    for h, b in enumerate(dblocks):
                        for k, (t, base) in enumerate(tiles):
                            r0, p = ROW_TILES[t]
                            kw = {} if t1 is None else {
                                "tile_position": (base, 0)}
                            psd = psD.tile([128, BLK], F32, tag="psd")
                            nc.tensor.matmul(
                                out=psd[:p, :],
                                lhsT=aet_b[base : base + 64, r0 : r0 + p],
                                rhs=et_b[base : base + 64,
                                         b * BLK : (b + 1) * BLK],
                                start=True, stop=True, **kw,
                            )
                            nc.vector.tensor_scalar(
                                out=its[k][:p, h * BLK : (h + 1) * BLK],
                                in0=psd[:p, :],
                                scalar1=b_rows[:p, t : t + 1],
                                scalar2=0.0,
                                op0=op_add, op1=op_max,
                            )
                    for k, (t, base) in enumerate(tiles):
                        r0, p = ROW_TILES[t]
                        slot = t * NSLOT + 6 + g
                        # packed-bf16 tree-add (2 elem/cycle) + short reduce
                        w = nd * BLK
                        v = its[k][:p, 0:w].bitcast(BF16)
                        h1 = tadd.tile([128, BLK], BF16, tag="h1")
                        nc.vector.tensor_add(
                            h1[:p, 0 : w // 2], v[:, 0 : w // 2],
                            v[:, w // 2 :],
                        )
                        h2 = tadd.tile([128, BLK // 2], BF16, tag="h2")
                        nc.vector.tensor_add(
                            h2[:p, 0 : w // 4], h1[:p, 0 : w // 4],
                            h1[:p, w // 4 : w // 2],
                        )
                        nc.vector.tensor_reduce(
                            out=dsums[:p, slot : slot + 1],
                            in_=h2[:p, 0 : w // 4],
                            axis=mybir.AxisListType.X,
                            op=op_add,
                        )

            # Alternating 15A/10D and 18A/7D pairs balances ACT vs DVE
            # busy time (measured best at ~155 us span)
            for pair in range(6):
                emit_tile_pair(2 * pair, 2 * pair + 1,
                               PLAN_U)
            emit_tile_pair(12, None, PLAN_U)

            # ---- tail: per-row loss and core-level sum ------------------
            den = small.tile([128, NRT], F32)
            nc.vector.tensor_reduce(
                out=den[:],
                in_=dsums[:].rearrange("p (t s) -> p t s", s=NSLOT),
                axis=mybir.AxisListType.X,
                op=op_add,
            )
            # scrub int32-saturation NaNs (saturated rows) to a huge finite
            den2 = small.tile([128, NRT], F32)
            nc.vector.tensor_scalar_min(out=den2[:], in0=den[:], scalar1=DEN_CAP)
            den_eps = small.tile([128, NRT], F32)
            nc.vector.scalar_tensor_tensor(
                out=den_eps[:], in0=den2[:], scalar=EPS, in1=e1_sb[:],
                op0=op_add, op1=op_add,
            )
            recip = small.tile([128, NRT], F32)
            nc.vector.reciprocal(out=recip[:], in_=den_eps[:])
            ratio = small.tile([128, NRT], F32)
            nc.vector.tensor_mul(ratio[:], e1_sb[:], recip[:])
            rateps = small.tile([128, NRT], F32)
            nc.vector.tensor_scalar_add(out=rateps[:], in0=ratio[:], scalar1=EPS)
            nc.vector.memset(rateps[64:128, NRT - 1 : NRT], 1.0)
            logv = small.tile([128, NRT], F32)
            lsum = small.tile([128, 1], F32)
            nc.scalar.activation(
                out=logv[:], in_=rateps[:], func=ln_f, accum_out=lsum[:]
            )
            tot_ps = psD.tile([128, BLK], F32, tag="psd")
            nc.tensor.matmul(
                out=tot_ps[0:1, 0:1], lhsT=lsum[:], rhs=ones_p[:],
                start=True, stop=True,
            )
            tot_sb = small.tile([1, 1], F32)
            nc.vector.tensor_copy(out=tot_sb[:], in_=tot_ps[0:1, 0:1])
            nc.sync.dma_start(out=out[:], in_=tot_sb[:])

    if not nc.is_finalized():
        nc.finalize()
    return nc


_NC_CACHE: list = []


def _get_nc() -> bass.Bass:
    if not _NC_CACHE:
        _NC_CACHE.append(_build())
    return _NC_CACHE[0]


_RUNNER_CACHE: list = []


def _get_runner():
    """Build the sharded PJRT executable once and reuse it across calls."""
    if _RUNNER_CACHE:
        return _RUNNER_CACHE[0]

    import jax
    import numpy as _np
    from jax.sharding import Mesh, PartitionSpec
    from jax.experimental.shard_map import shard_map
    from concourse import mybir as _mybir
    from concourse.bass2jax import (
        _bass_exec_p,
        install_neuronx_cc_hook,
        partition_id_tensor,
    )

    nc = _get_nc()
    install_neuronx_cc_hook()
    partition_name = nc.partition_id_tensor.name if nc.partition_id_tensor else None

    in_names, out_names, out_avals, zero_outs = [], [], [], []
    for alloc in nc.m.functions[0].allocations:
        if not isinstance(alloc, _mybir.MemoryLocationSet):
            continue
        name = alloc.memorylocations[0].name
        if alloc.kind == "ExternalInput":
            if name != partition_name:
                in_names.append(name)
        elif alloc.kind == "ExternalOutput":
            shape = tuple(alloc.tensor_shape)
            dtype = _mybir.dt.np(alloc.dtype)
            out_names.append(name)
            out_avals.append(jax.core.ShapedArray(shape, dtype))
            zero_outs.append(_np.zeros(shape, dtype))
    n_params = len(in_names)
    n_outs = len(out_avals)
    all_in_names = list(in_names) + list(out_names)
    if partition_name is not None:
        all_in_names.append(partition_name)

    def _body(*args):
        operands = list(args)
        if partition_name is not None:
            operands.append(partition_id_tensor())
        outs = _bass_exec_p.bind(
            *operands,
            out_avals=tuple(out_avals),
            in_names=tuple(all_in_names),
            out_names=tuple(out_names),
            lowering_input_output_aliases=(),
            sim_require_finite=False,
            sim_require_nnan=False,
            nc=nc,
        )
        return tuple(outs)

    devices = jax.devices()[:NCORES]
    mesh = Mesh(_np.asarray(devices), ("core",))
    spec_of = {
        "ae_tb": PartitionSpec("core"),
        "bias_in": PartitionSpec("core"),
        "e1_in": PartitionSpec("core"),
        "e_tb": PartitionSpec(),
    }
    in_specs = tuple(spec_of[nm] for nm in in_names) + (
        PartitionSpec("core"),
    ) * n_outs
    out_specs = (PartitionSpec("core"),) * n_outs
    donate = tuple(range(n_params, n_params + n_outs))
    sharded = jax.jit(
        shard_map(
            _body, mesh=mesh, in_specs=in_specs, out_specs=out_specs, check_rep=False
        ),
        donate_argnums=donate,
        keep_unused=True,
    )

    state = (sharded, in_names, out_names, out_avals, zero_outs)
    _RUNNER_CACHE.append(state)
    return state


def _to_bf16(x: np.ndarray):
    import ml_dtypes

    return x.astype(ml_dtypes.bfloat16)


def _prep_feeds(proj_main, proj_ema):
    """Full inputs -> (ae stacked, bias stacked, e1 stacked, e2 bf16)."""
    import ml_dtypes

    pm = np.ascontiguousarray(np.asarray(proj_main, dtype=np.float32))
    pe = np.ascontiguousarray(np.asarray(proj_ema, dtype=np.float32))
    at_full = np.ascontiguousarray(pm.transpose(1, 0, 2, 3).reshape(C, N))
    et_full = np.ascontiguousarray(pe.transpose(1, 0, 2, 3).reshape(C, N))

    pos = (at_full * et_full).sum(axis=0, dtype=np.float32)  # (N,) raw dots

    # per-row shift m' = pos + MARGIN (fp32); ACT bias = -10*m'
    bias_act = (-10.0 * (pos + np.float32(MARGIN))).astype(np.float32)
    e1 = np.exp(10.0 * pos.astype(np.float64)
                + bias_act.astype(np.float64)).astype(np.float32)

    a_scaled = _to_bf16(at_full * np.float32(ALPHA))     # (64, N) bf16
    e_b16 = _to_bf16(et_full)                            # (64, N) bf16
    ae_full = np.empty((128, N), dtype=ml_dtypes.bfloat16)
    ae_full[0:64] = a_scaled
    ae_full[64:128] = a_scaled
    e2 = np.empty((128, N), dtype=ml_dtypes.bfloat16)
    e2[0:64] = e_b16
    e2[64:128] = e_b16

    def layout_rows(v, pad=0.0):
        vp = np.full(NCORES * NRT * 128, pad, dtype=np.float32)
        for core in range(NCORES):
            vp[core * NRT * 128 : core * NRT * 128 + R] = v[
                core * R : (core + 1) * R
            ]
        return np.ascontiguousarray(
            vp.reshape(NCORES, NRT, 128).transpose(0, 2, 1).reshape(
                NCORES * 128, NRT
            )
        )

    bias_in = layout_rows(bias_act)
    e1_in = layout_rows(e1, pad=0.0)
    ae_sh = np.ascontiguousarray(
        np.asarray(ae_full).reshape(128, NCORES, R).transpose(1, 0, 2).reshape(
            NCORES * 128, R
        )
    )
    return ae_sh, bias_in, e1_in, np.ascontiguousarray(e2)


def _trace_in_maps(np_inputs):
    """Per-core input dicts for run_bass_kernel_spmd (trace harness)."""
    ae_sh, bias_in, e1_in, e2 = _prep_feeds(
        np_inputs["proj_main"], np_inputs["proj_ema"]
    )
    maps = []
    for core in range(NCORES):
        maps.append(
            {
                "ae_tb": np.ascontiguousarray(
                    ae_sh[core * 128 : (core + 1) * 128]
                ),
                "bias_in": np.ascontiguousarray(
                    bias_in[core * 128 : (core + 1) * 128]
                ),
                "e1_in": np.ascontiguousarray(
                    e1_in[core * 128 : (core + 1) * 128]
                ),
                "e_tb": e2,
            }
        )
    return maps


def kernel(proj_main, proj_ema, label_main, label_ema, patch_num):
    # labels / patch_num never influence the loss; only the projections do.
    ae_sh, bias_in, e1_in, e2 = _prep_feeds(proj_main, proj_ema)

    sharded, in_names, out_names, out_avals, zero_outs = _get_runner()
    feed = {"ae_tb": ae_sh, "bias_in": bias_in, "e1_in": e1_in, "e_tb": e2}
    args = [feed[nm] for nm in in_names]
    args += [
        np.zeros((NCORES * z.shape[0], *z.shape[1:]), z.dtype) for z in zero_outs
    ]
    out_arrs = sharded(*args)
    outs = np.asarray(out_arrs[out_names.index("out")]).reshape(NCORES)
    return np.float32(-float(outs.sum()) / N)


if __name__ == "__main__":
    _build()
    print("build OK")


# revision 47
# speedup vs baseline: 1.2952x; 1.0239x over previous
"""Trainium2 Bass kernel for nn_DenseContrastive (dense contrastive loss).

Math (per the fused reference):
    A = anchors (N, c), E = ema features (N, c), N = 12800, c = 64
    pos_i   = (A_i . E_i) / TEMP
    l_ij    = (A_i . E_j) / TEMP
    den_i   = sum_j exp(l_ij - s_i)            (s_i = per-row shift)
    r_i     = e1_i / (den_i + EPS + e1_i),  e1_i = exp(pos_i - s_i)
    loss_i  = -log(r_i + EPS);   out = mean_i loss_i

Shift choice (the structural win): s_i = pos_i + MARGIN, known on the
host -- no row-max pass.  The denominator always contains the diagonal
term exp(-MARGIN), so r <= ~1/2; if any exp overflows (row max > ~88
logits above s) then den saturates and loss = -log(EPS), which is
exactly what the reference computes for such a row (its softmax ratio
underflows below EPS).  Verified to rel err ~5e-7 vs the fp32 reference.

PE tiling (the throughput win): K = 64 contraction channels only fills
half the 128-row PE array, and a 64-partition moving stream runs at half
rate.  A and E are duplicated into both partition halves and row tiles
are processed in PAIRS: tile_position (0,0) computes row-tile t0 on PE
rows 0-63 while (64,0) computes t1 on rows 64-127, concurrently
(measured ~3x matmul throughput vs unpaired).

exp work is split across ACT and DVE per row tile (25 x 512 blocks,
15/10 and 18/7 on alternating pairs to balance engine busy time):
  ACT: table exp of scale*PSUM + bias_row (per-partition AP);
      accum_out gives the row sums for free.
  DVE: bf16 Schraudolph -- i16 = rne(max(PSUM + Brow, 0)), whose bit
      pattern read as bf16 is ~exp(10x) (ALPHA = 10*log2e*2^7 is folded
      into A; Brow = B16 - ALPHA*(pos+MARGIN), a per-partition scalar
      AP, folds the exponent bias and the row shift; the high side
      saturates the int16 convert to 0x7FFF = bf16 NaN, which a final
      min(den, 3e38) scrubs -- DVE min takes the non-NaN operand, and
      such rows are saturated anyway).  Row sums run as a packed-bf16
      tensor_tensor ADD tree (2 elem/cycle) plus a short 1x reduce.
e1 is computed on the host from the same shift, so numerator and
denominator shifts cancel exactly.

Sharding: N rows split across 8 cores (1600 each); E' (128 x 12800 bf16,
duplicated halves) replicated per core.  Core returns sum_i log(r_i+EPS);
host combines: loss = -sum/N.
"""

import sys

for _p in ("/opt/trn_rl_repo",):
    if _p not in sys.path:
        sys.path.insert(0, _p)

import numpy as np

import concourse.bass as bass
import concourse.bacc as bacc
import concourse.tile as tile
from concourse import mybir

TEMP = 0.1
EPS = 1e-8
B_, C, H, W = 2, 64, 80, 80
N = B_ * H * W           # 12800 anchors
NCORES = 8
R = N // NCORES          # 1600 rows per core
BLK = 512                # logit columns per PSUM bank
NBLK = N // BLK          # 25
MARGIN = 0.5             # raw-dot units; e1 ~ e^-5

LOG2E = 1.4426950408889634
# bf16-domain Schraudolph: exp(10*x) ~ bitcast16 of round(ALPHA*x + Brow)
ALPHA = np.float32(10.0 * LOG2E * (1 << 7))           # 1846.6497
SCHRAUD_C = 486408.0 / 65536.0                        # ~7.42
BCONST = np.float32(127.0 * (1 << 7) - SCHRAUD_C)     # ~16248.58
ACT_SCALE = np.float32(10.0 / float(ALPHA))
DEN_CAP = 3.0e38

F32 = mybir.dt.float32
I16 = mybir.dt.int16
BF16 = mybir.dt.bfloat16

# 1600 rows -> 6 pairs of 128-row tiles + one 64-row tile
ROW_TILES = [(i * 128, 128) for i in range(12)] + [(1536, 64)]
NRT = len(ROW_TILES)
NSLOT = 11               # up to 6 ACT accum slots + 5 reduce slots

# Alternating per-pair block plans balance ACT vs DVE at ~16.6/8.4:
# PLAN_A = 15 ACT / 10 DVE, PLAN_B = 18 ACT / 7 DVE.  (A uniform 17/8
# plan and a 2:4 mix both measured worse -- the alternating pair mix
# is the empirical optimum.)
PLAN_A = [
    ((0, 1, 2), (3, 4)),
    ((5, 6, 7), (8, 9)),
    ((10, 11, 12), (13, 14)),
    ((15, 16, 17), (18, 19)),
    ((20, 21, 22), (23, 24)),
]
PLAN_B = [
    ((0, 1, 2), (3, 4)),
    ((5, 6, 7), (8, 9)),
    ((10, 11, 12), (13, 14)),
    ((15, 16, 17), (18,)),
    ((19, 20, 21), ()),
    ((22, 23, 24), ()),
]


def _build() -> bass.Bass:
    nc = bacc.Bacc("TRN2", target_bir_lowering=False)
    ae_tb = nc.declare_dram_parameter("ae_tb", [128, R], BF16, isOutput=False)
    e_tb = nc.declare_dram_parameter("e_tb", [128, N], BF16, isOutput=False)
    bias_in = nc.declare_dram_parameter("bias_in", [128, NRT], F32, isOutput=False)
    e1_in = nc.declare_dram_parameter("e1_in", [128, NRT], F32, isOutput=False)
    out = nc.declare_dram_parameter("out", [1, 1], F32, isOutput=True)

    exp_f = mybir.ActivationFunctionType.Exp
    ln_f = mybir.ActivationFunctionType.Ln
    op_add = mybir.AluOpType.add
    op_max = mybir.AluOpType.max
    op_min = mybir.AluOpType.min
    op_mult = mybir.AluOpType.mult

    with tile.TileContext(nc) as tc:
        with (
            tc.tile_pool(name="big", bufs=1) as big,
            tc.tile_pool(name="small", bufs=1) as small,
            tc.tile_pool(name="trash", bufs=2) as trash,
            tc.tile_pool(name="intb", bufs=4) as intb,
            tc.tile_pool(name="tadd", bufs=2) as tadd,
            tc.tile_pool(name="psA", bufs=2, space="PSUM") as psA,
            tc.tile_pool(name="psD", bufs=2, space="PSUM") as psD,
        ):
            # ---- resident SBUF data -------------------------------------
            et_b = big.tile([128, N], BF16)     # E' dup halves (3.3 MB)
            aet_b = big.tile([128, R], BF16)    # A' shard, dup halves
            bias_sb = small.tile([128, NRT], F32)
            e1_sb = small.tile([128, NRT], F32)
            nc.sync.dma_start(out=aet_b[:], in_=ae_tb[:])
            nc.sync.dma_start(out=bias_sb[:], in_=bias_in[:])
            nc.sync.dma_start(out=e1_sb[:], in_=e1_in[:])
            for k in range(8):
                s = slice(k * (N // 8), (k + 1) * (N // 8))
                nc.sync.dma_start(out=et_b[:, s], in_=e_tb[:, s])

            ones_p = small.tile([128, 1], F32)
            nc.vector.memset(ones_p, 1.0)

            # Schraudolph per-row exponent bias: Brow = (ALPHA/10)*bias + B
            b_rows = small.tile([128, NRT], F32)
            nc.vector.tensor_scalar(
                out=b_rows[:], in0=bias_sb[:],
                scalar1=float(ALPHA) / 10.0, scalar2=float(BCONST),
                op0=op_mult, op1=op_add,
            )
            dsums = small.tile([128, NRT * NSLOT], F32)
            nc.vector.memset(dsums[:], 0.0)

            def emit_tile_pair(t0, t1, plan):
                """Row tiles t0 (PE rows 0-63) and t1 (rows 64-127), paired.
                t1 may be None (odd tail tile, unpaired)."""
                tiles = [(t0, 0)] + ([(t1, 64)] if t1 is not None else [])
                for g, (blocks, dblocks) in enumerate(plan):
                    psas, its = [], []
                    for (t, base) in tiles:
                        psas.append(
                            psA.tile([128, 3 * BLK], F32, tag="psa",
                                     name=f"psa_{t}_{g}")
                        )
                    for j, b in enumerate(blocks):
                        for k, (t, base) in enumerate(tiles):
                            r0, p = ROW_TILES[t]
                            kw = {} if t1 is None else {
                                "tile_position": (base, 0)}
                            nc.tensor.matmul(
                                out=psas[k][:p, j * BLK : (j + 1) * BLK],
                                lhsT=aet_b[base : base + 64, r0 : r0 + p],
                                rhs=et_b[base : base + 64,
                                         b * BLK : (b + 1) * BLK],
                                start=True, stop=True, **kw,
                            )
                    nb = len(blocks)
                    for k, (t, base) in enumerate(tiles):
                        r0, p = ROW_TILES[t]
                        scr = trash.tile([128, 3 * BLK], BF16, tag="scr")
                        nc.scalar.activation(
                            out=scr[:p, 0 : nb * BLK],
                            in_=psas[k][:p, 0 : nb * BLK],
                            func=exp_f,
                            scale=float(ACT_SCALE),
                            bias=bias_sb[:p, t : t + 1],
                            accum_out=dsums[:p, t * NSLOT + g : t * NSLOT + g + 1],
                        )
                    if not dblocks:
                        continue
                    nd = len(dblocks)
                    for (t, base) in tiles:
                        its.append(
                            intb.tile([128, 2 * BLK], I16, tag="intb",
                                      name=f"intb_{t}_{g}")
                        )
                    for h, b in enumerate(dblocks):
                        for k, (t, base) in enumerate(tiles):
                            r0, p = ROW_TILES[t]
                            kw = {} if t1 is None else {
                                "tile_position": (base, 0)}
                            psd = psD.tile([128, BLK], F32, tag="psd")
                            nc.tensor.matmul(
                                out=psd[:p, :],
                                lhsT=aet_b[base : base + 64, r0 : r0 + p],
                                rhs=et_b[base : base + 64,
                                         b * BLK : (b + 1) * BLK],
                                start=True, stop=True, **kw,
                            )
                            nc.vector.tensor_scalar(
                                out=its[k][:p, h * BLK : (h + 1) * BLK],
                                in0=psd[:p, :],
                                scalar1=b_rows[:p, t : t + 1],
                                scalar2=0.0,
                                op0=op_add, op1=op_max,
                            )
                    for k, (t, base) in enumerate(tiles):
                        r0, p = ROW_TILES[t]
                        slot = t * NSLOT + 6 + g
                        # packed-bf16 tree-add (2 elem/cycle) + short reduce
                        w = nd * BLK
                        v = its[k][:p, 0:w].bitcast(BF16)
                        h1 = tadd.tile([128, BLK], BF16, tag="h1")
                        nc.vector.tensor_add(
                            h1[:p, 0 : w // 2], v[:, 0 : w // 2],
                            v[:, w // 2 :],
                        )
                        h2 = tadd.tile([128, BLK // 2], BF16, tag="h2")
                        nc.vector.tensor_add(
                            h2[:p, 0 : w // 4], h1[:p, 0 : w // 4],
                            h1[:p, w // 4 : w // 2],
                        )
                        nc.vector.tensor_reduce(
                            out=dsums[:p, slot : slot + 1],
                            in_=h2[:p, 0 : w // 4],
                            axis=mybir.AxisListType.X,
                            op=op_add,
                        )

            # Alternating 15A/10D and 18A/7D pairs balances ACT vs DVE
            # busy time (measured best at ~155 us span)
            for pair in range(6):
                emit_tile_pair(2 * pair, 2 * pair + 1,
                               PLAN_A if pair % 2 == 0 else PLAN_B)
            emit_tile_pair(12, None, PLAN_B)

            # ---- tail: per-row loss and core-level sum ------------------
            den = small.tile([128, NRT], F32)
            nc.vector.tensor_reduce(
                out=den[:],
                in_=dsums[:].rearrange("p (t s) -> p t s", s=NSLOT),
                axis=mybir.AxisListType.X,
                op=op_add,
            )
            # scrub int32-saturation NaNs (saturated rows) to a huge finite
            den2 = small.tile([128, NRT], F32)
            nc.vector.tensor_scalar_min(out=den2[:], in0=den[:], scalar1=DEN_CAP)
            den_eps = small.tile([128, NRT], F32)
            nc.vector.scalar_tensor_tensor(
                out=den_eps[:], in0=den2[:], scalar=EPS, in1=e1_sb[:],
                op0=op_add, op1=op_add,
            )
            recip = small.tile([128, NRT], F32)
            nc.vector.reciprocal(out=recip[:], in_=den_eps[:])
            ratio = small.tile([128, NRT], F32)
            nc.vector.tensor_mul(ratio[:], e1_sb[:], recip[:])
            rateps = small.tile([128, NRT], F32)
            nc.vector.tensor_scalar_add(out=rateps[:], in0=ratio[:], scalar1=EPS)
            nc.vector.memset(rateps[64:128, NRT - 1 : NRT], 1.0)
            logv = small.tile([128, NRT], F32)
            lsum = small.tile([128, 1], F32)
            nc.scalar.activation(
                out=logv[:], in_=rateps[:], func=ln_f, accum_out=lsum[:]
            )
            tot_ps = psD.tile([128, BLK], F32, tag="psd")
            nc.tensor.matmul(
                out=tot_ps[0:1, 0:1], lhsT=lsum[:], rhs=ones_p[:],
                start=True, stop=True,
            )
            tot_sb = small.tile([1, 1], F32)
            nc.vector.tensor_copy(out=tot_sb[:], in_=tot_ps[0:1, 0:1])
            nc.sync.dma_start(out=out[:], in_=tot_sb[:])

    if not nc.is_finalized():
        nc.finalize()
    return nc


_NC_CACHE: list = []


def _get_nc() -> bass.Bass:
    if not _NC_CACHE:
        _NC_CACHE.append(_build())
    return _NC_CACHE[0]


_RUNNER_CACHE: list = []


def _get_runner():
    """Build the sharded PJRT executable once and reuse it across calls."""
    if _RUNNER_CACHE:
        return _RUNNER_CACHE[0]

    import jax
    import numpy as _np
    from jax.sharding import Mesh, PartitionSpec
    from jax.experimental.shard_map import shard_map
    from concourse import mybir as _mybir
    from concourse.bass2jax import (
        _bass_exec_p,
        install_neuronx_cc_hook,
        partition_id_tensor,
    )

    nc = _get_nc()
    install_neuronx_cc_hook()
    partition_name = nc.partition_id_tensor.name if nc.partition_id_tensor else None

    in_names, out_names, out_avals, zero_outs = [], [], [], []
    for alloc in nc.m.functions[0].allocations:
        if not isinstance(alloc, _mybir.MemoryLocationSet):
            continue
        name = alloc.memorylocations[0].name
        if alloc.kind == "ExternalInput":
            if name != partition_name:
                in_names.append(name)
        elif alloc.kind == "ExternalOutput":
            shape = tuple(alloc.tensor_shape)
            dtype = _mybir.dt.np(alloc.dtype)
            out_names.append(name)
            out_avals.append(jax.core.ShapedArray(shape, dtype))
            zero_outs.append(_np.zeros(shape, dtype))
    n_params = len(in_names)
    n_outs = len(out_avals)
    all_in_names = list(in_names) + list(out_names)
    if partition_name is not None:
        all_in_names.append(partition_name)

    def _body(*args):
        operands = list(args)
        if partition_name is not None:
            operands.append(partition_id_tensor())
        outs = _bass_exec_p.bind(
            *operands,
            out_avals=tuple(out_avals),
            in_names=tuple(all_in_names),
            out_names=tuple(out_names),
            lowering_input_output_aliases=(),
            sim_require_finite=False,
            sim_require_nnan=False,
            nc=nc,
        )
        return tuple(outs)

    devices = jax.devices()[:NCORES]
    mesh = Mesh(_np.asarray(devices), ("core",))
    spec_of = {
        "ae_tb": PartitionSpec("core"),
        "bias_in": PartitionSpec("core"),
        "e1_in": PartitionSpec("core"),
        "e_tb": PartitionSpec(),
    }
    in_specs = tuple(spec_of[nm] for nm in in_names) + (
        PartitionSpec("core"),
    ) * n_outs
    out_specs = (PartitionSpec("core"),) * n_outs
    donate = tuple(range(n_params, n_params + n_outs))
    sharded = jax.jit(
        shard_map(
            _body, mesh=mesh, in_specs=in_specs, out_specs=out_specs, check_rep=False
        ),
        donate_argnums=donate,
        keep_unused=True,
    )

    state = (sharded, in_names, out_names, out_avals, zero_outs)
    _RUNNER_CACHE.append(state)
    return state


def _to_bf16(x: np.ndarray):
    import ml_dtypes

    return x.astype(ml_dtypes.bfloat16)


def _prep_feeds(proj_main, proj_ema):
    """Full inputs -> (ae stacked, bias stacked, e1 stacked, e2 bf16)."""
    import ml_dtypes

    pm = np.ascontiguousarray(np.asarray(proj_main, dtype=np.float32))
    pe = np.ascontiguousarray(np.asarray(proj_ema, dtype=np.float32))
    at_full = np.ascontiguousarray(pm.transpose(1, 0, 2, 3).reshape(C, N))
    et_full = np.ascontiguousarray(pe.transpose(1, 0, 2, 3).reshape(C, N))

    pos = (at_full * et_full).sum(axis=0, dtype=np.float32)  # (N,) raw dots

    # per-row shift m' = pos + MARGIN (fp32); ACT bias = -10*m'
    bias_act = (-10.0 * (pos + np.float32(MARGIN))).astype(np.float32)
    e1 = np.exp(10.0 * pos.astype(np.float64)
                + bias_act.astype(np.float64)).astype(np.float32)

    a_scaled = _to_bf16(at_full * np.float32(ALPHA))     # (64, N) bf16
    e_b16 = _to_bf16(et_full)                            # (64, N) bf16
    ae_full = np.empty((128, N), dtype=ml_dtypes.bfloat16)
    ae_full[0:64] = a_scaled
    ae_full[64:128] = a_scaled
    e2 = np.empty((128, N), dtype=ml_dtypes.bfloat16)
    e2[0:64] = e_b16
    e2[64:128] = e_b16

    def layout_rows(v, pad=0.0):
        vp = np.full(NCORES * NRT * 128, pad, dtype=np.float32)
        for core in range(NCORES):
            vp[core * NRT * 128 : core * NRT * 128 + R] = v[
                core * R : (core + 1) * R
            ]
        return np.ascontiguousarray(
            vp.reshape(NCORES, NRT, 128).transpose(0, 2, 1).reshape(
                NCORES * 128, NRT
            )
        )

    bias_in = layout_rows(bias_act)
    e1_in = layout_rows(e1, pad=0.0)
    ae_sh = np.ascontiguousarray(
        np.asarray(ae_full).reshape(128, NCORES, R).transpose(1, 0, 2).reshape(
            NCORES * 128, R
        )
    )
    return ae_sh, bias_in, e1_in, np.ascontiguousarray(e2)


def _trace_in_maps(np_inputs):
    """Per-core input dicts for run_bass_kernel_spmd (trace harness)."""
    ae_sh, bias_in, e1_in, e2 = _prep_feeds(
        np_inputs["proj_main"], np_inputs["proj_ema"]
    )
    maps = []
    for core in range(NCORES):
        maps.append(
            {
                "ae_tb": np.ascontiguousarray(
                    ae_sh[core * 128 : (core + 1) * 128]
                ),
                "bias_in": np.ascontiguousarray(
                    bias_in[core * 128 : (core + 1) * 128]
                ),
                "e1_in": np.ascontiguousarray(
                    e1_in[core * 128 : (core + 1) * 128]
                ),
                "e_tb": e2,
            }
        )
    return maps


def kernel(proj_main, proj_ema, label_main, label_ema, patch_num):
    # labels / patch_num never influence the loss; only the projections do.
    ae_sh, bias_in, e1_in, e2 = _prep_feeds(proj_main, proj_ema)

    sharded, in_names, out_names, out_avals, zero_outs = _get_runner()
    feed = {"ae_tb": ae_sh, "bias_in": bias_in, "e1_in": e1_in, "e_tb": e2}
    args = [feed[nm] for nm in in_names]
    args += [
        np.zeros((NCORES * z.shape[0], *z.shape[1:]), z.dtype) for z in zero_outs
    ]
    out_arrs = sharded(*args)
    outs = np.asarray(out_arrs[out_names.index("out")]).reshape(NCORES)
    return np.float32(-float(outs.sum()) / N)


if __name__ == "__main__":
    _build()
    print("build OK")


# revision 49
# speedup vs baseline: 1.2961x; 1.0007x over previous
"""Trainium2 Bass kernel for nn_DenseContrastive (dense contrastive loss).

Math (per the fused reference):
    A = anchors (N, c), E = ema features (N, c), N = 12800, c = 64
    pos_i   = (A_i . E_i) / TEMP
    l_ij    = (A_i . E_j) / TEMP
    den_i   = sum_j exp(l_ij - s_i)            (s_i = per-row shift)
    r_i     = e1_i / (den_i + EPS + e1_i),  e1_i = exp(pos_i - s_i)
    loss_i  = -log(r_i + EPS);   out = mean_i loss_i

Shift choice (the structural win): s_i = pos_i + MARGIN, known on the
host -- no row-max pass.  The denominator always contains the diagonal
term exp(-MARGIN), so r <= ~1/2; if any exp overflows (row max > ~88
logits above s) then den saturates and loss = -log(EPS), which is
exactly what the reference computes for such a row (its softmax ratio
underflows below EPS).  Verified to rel err ~5e-7 vs the fp32 reference.

PE tiling (the throughput win): K = 64 contraction channels only fills
half the 128-row PE array, and a 64-partition moving stream runs at half
rate.  A and E are duplicated into both partition halves and row tiles
are processed in PAIRS: tile_position (0,0) computes row-tile t0 on PE
rows 0-63 while (64,0) computes t1 on rows 64-127, concurrently
(measured ~3x matmul throughput vs unpaired).

exp work is split across ACT and DVE per row tile (25 x 512 blocks,
15/10 and 18/7 on alternating pairs to balance engine busy time):
  ACT: table exp of scale*PSUM + bias_row (per-partition AP);
      accum_out gives the row sums for free.
  DVE: bf16 Schraudolph -- i16 = rne(max(PSUM + Brow, 0)), whose bit
      pattern read as bf16 is ~exp(10x) (ALPHA = 10*log2e*2^7 is folded
      into A; Brow = B16 - ALPHA*(pos+MARGIN), a per-partition scalar
      AP, folds the exponent bias and the row shift; the high side
      saturates the int16 convert to 0x7FFF = bf16 NaN, which a final
      min(den, 3e38) scrubs -- DVE min takes the non-NaN operand, and
      such rows are saturated anyway).  Row sums run as a packed-bf16
      tensor_tensor ADD tree (2 elem/cycle) plus a short 1x reduce.
e1 is computed on the host from the same shift, so numerator and
denominator shifts cancel exactly.

Sharding: N rows split across 8 cores (1600 each); E' (128 x 12800 bf16,
duplicated halves) replicated per core.  Core returns sum_i log(r_i+EPS);
host combines: loss = -sum/N.
"""

import sys

for _p in ("/opt/trn_rl_repo",):
    if _p not in sys.path:
        sys.path.insert(0, _p)

import numpy as np

import concourse.bass as bass
import concourse.bacc as bacc
import concourse.tile as tile
from concourse import mybir

TEMP = 0.1
EPS = 1e-8
B_, C, H, W = 2, 64, 80, 80
N = B_ * H * W           # 12800 anchors
NCORES = 8
R = N // NCORES          # 1600 rows per core
BLK = 512                # logit columns per PSUM bank
NBLK = N // BLK          # 25
MARGIN = 0.5             # raw-dot units; e1 ~ e^-5

LOG2E = 1.4426950408889634
# bf16-domain Schraudolph: exp(10*x) ~ bitcast16 of round(ALPHA*x + Brow)
ALPHA = np.float32(10.0 * LOG2E * (1 << 7))           # 1846.6497
SCHRAUD_C = 486408.0 / 65536.0                        # ~7.42
BCONST = np.float32(127.0 * (1 << 7) - SCHRAUD_C)     # ~16248.58
ACT_SCALE = np.float32(10.0 / float(ALPHA))
DEN_CAP = 3.0e38

F32 = mybir.dt.float32
I16 = mybir.dt.int16
BF16 = mybir.dt.bfloat16

# 1600 rows -> 6 pairs of 128-row tiles + one 64-row tile
ROW_TILES = [(i * 128, 128) for i in range(12)] + [(1536, 64)]
NRT = len(ROW_TILES)
NSLOT = 11               # up to 6 ACT accum slots + 5 reduce slots

# Alternating per-pair block plans balance ACT vs DVE at ~16.6/8.4:
# PLAN_A = 15 ACT / 10 DVE, PLAN_B = 18 ACT / 7 DVE.  (A uniform 17/8
# plan and a 2:4 mix both measured worse -- the alternating pair mix
# is the empirical optimum.)
PLAN_A = [
    ((0, 1, 2), (3, 4)),
    ((5, 6, 7), (8, 9)),
    ((10, 11, 12), (13, 14)),
    ((15, 16, 17), (18, 19)),
    ((20, 21, 22), (23, 24)),
]
PLAN_B = [
    ((0, 1, 2), (3, 4)),
    ((5, 6, 7), (8, 9)),
    ((10, 11, 12), (13, 14)),
    ((15, 16, 17), (18,)),
    ((19, 20, 21), ()),
    ((22, 23, 24), ()),
]


def _build() -> bass.Bass:
    nc = bacc.Bacc("TRN2", target_bir_lowering=False)
    ae_tb = nc.declare_dram_parameter("ae_tb", [128, R], BF16, isOutput=False)
    e_tb = nc.declare_dram_parameter("e_tb", [128, N], BF16, isOutput=False)
    bias_in = nc.declare_dram_parameter("bias_in", [128, NRT], F32, isOutput=False)
    e1_in = nc.declare_dram_parameter("e1_in", [128, NRT], F32, isOutput=False)
    out = nc.declare_dram_parameter("out", [1, 1], F32, isOutput=True)

    exp_f = mybir.ActivationFunctionType.Exp
    ln_f = mybir.ActivationFunctionType.Ln
    op_add = mybir.AluOpType.add
    op_max = mybir.AluOpType.max
    op_min = mybir.AluOpType.min
    op_mult = mybir.AluOpType.mult

    with tile.TileContext(nc) as tc:
        with (
            tc.tile_pool(name="big", bufs=1) as big,
            tc.tile_pool(name="small", bufs=1) as small,
            tc.tile_pool(name="trash", bufs=2) as trash,
            tc.tile_pool(name="intb", bufs=4) as intb,
            tc.tile_pool(name="tadd", bufs=2) as tadd,
            tc.tile_pool(name="psA", bufs=2, space="PSUM") as psA,
            tc.tile_pool(name="psD", bufs=2, space="PSUM") as psD,
        ):
            # ---- resident SBUF data -------------------------------------
            et_b = big.tile([128, N], BF16)     # E' dup halves (3.3 MB)
            aet_b = big.tile([128, R], BF16)    # A' shard, dup halves
            bias_sb = small.tile([128, NRT], F32)
            e1_sb = small.tile([128, NRT], F32)
            nc.sync.dma_start(out=aet_b[:], in_=ae_tb[:])
            nc.sync.dma_start(out=bias_sb[:], in_=bias_in[:])
            nc.sync.dma_start(out=e1_sb[:], in_=e1_in[:])
            for k in range(8):
                s = slice(k * (N // 8), (k + 1) * (N // 8))
                nc.sync.dma_start(out=et_b[:, s], in_=e_tb[:, s])

            ones_p = small.tile([128, 1], F32)
            nc.vector.memset(ones_p, 1.0)

            # Schraudolph per-row exponent bias: Brow = (ALPHA/10)*bias + B
            b_rows = small.tile([128, NRT], F32)
            nc.vector.tensor_scalar(
                out=b_rows[:], in0=bias_sb[:],
                scalar1=float(ALPHA) / 10.0, scalar2=float(BCONST),
                op0=op_mult, op1=op_add,
            )
            dsums = small.tile([128, NRT * NSLOT], F32)
            nc.vector.memset(dsums[:], 0.0)

            def emit_tile_pair(t0, t1, plan):
                """Row tiles t0 (PE rows 0-63) and t1 (rows 64-127), paired.
                t1 may be None (odd tail tile, unpaired)."""
                tiles = [(t0, 0)] + ([(t1, 64)] if t1 is not None else [])
                for g, (blocks, dblocks) in enumerate(plan):
                    psas, its = [], []
                    for (t, base) in tiles:
                        psas.append(
                            psA.tile([128, 3 * BLK], F32, tag="psa",
                                     name=f"psa_{t}_{g}")
                        )
                    for j, b in enumerate(blocks):
                        for k, (t, base) in enumerate(tiles):
                            r0, p = ROW_TILES[t]
                            kw = {} if t1 is None else {
                                "tile_position": (base, 0)}
                            nc.tensor.matmul(
                                out=psas[k][:p, j * BLK : (j + 1) * BLK],
                                lhsT=aet_b[base : base + 64, r0 : r0 + p],
                                rhs=et_b[base : base + 64,
                                         b * BLK : (b + 1) * BLK],
                                start=True, stop=True, **kw,
                            )
                    nb = len(blocks)
                    for k, (t, base) in enumerate(tiles):
                        r0, p = ROW_TILES[t]
                        scr = trash.tile([128, 3 * BLK], BF16, tag="scr")
                        nc.scalar.activation(
                            out=scr[:p, 0 : nb * BLK],
                            in_=psas[k][:p, 0 : nb * BLK],
                            func=exp_f,
                            scale=float(ACT_SCALE),
                            bias=bias_sb[:p, t : t + 1],
                            accum_out=dsums[:p, t * NSLOT + g : t * NSLOT + g + 1],
                        )
                    if not dblocks:
                        continue
                    nd = len(dblocks)
                    for (t, base) in tiles:
                        its.append(
                            intb.tile([128, 2 * BLK], I16, tag="intb",
                                      name=f"intb_{t}_{g}")
                        )
                    for h, b in enumerate(dblocks):
                        for k, (t, base) in enumerate(tiles):
                            r0, p = ROW_TILES[t]
                            kw = {} if t1 is None else {
                                "tile_position": (base, 0)}
                            psd = psD.tile([128, BLK], F32, tag="psd")
                            nc.tensor.matmul(
                                out=psd[:p, :],
                                lhsT=aet_b[base : base + 64, r0 : r0 + p],
                                rhs=et_b[base : base + 64,
                                         b * BLK : (b + 1) * BLK],
                                start=True, stop=True, **kw,
                            )
                            nc.vector.tensor_scalar(
                                out=its[k][:p, h * BLK : (h + 1) * BLK],
                                in0=psd[:p, :],
                                scalar1=b_rows[:p, t : t + 1],
                                scalar2=0.0,
                                op0=op_add, op1=op_max,
                            )
                    for k, (t, base) in enumerate(tiles):
                        r0, p = ROW_TILES[t]
                        slot = t * NSLOT + 6 + g
                        # packed-bf16 tree-add (2 elem/cycle) + short reduce
                        # (tensor_tensor_reduce / tensor_mask_reduce would
                        # fuse this, but custom-DVE ucode ops crash the
                        # exec unit in this runtime)
                        w = nd * BLK
                        v = its[k][:p, 0:w].bitcast(BF16)
                        h1 = tadd.tile([128, BLK], BF16, tag="h1")
                        nc.vector.tensor_add(
                            h1[:p, 0 : w // 2], v[:, 0 : w // 2],
                            v[:, w // 2 :],
                        )
                        h2 = tadd.tile([128, BLK // 2], BF16, tag="h2")
                        nc.vector.tensor_add(
                            h2[:p, 0 : w // 4], h1[:p, 0 : w // 4],
                            h1[:p, w // 4 : w // 2],
                        )
                        nc.vector.tensor_reduce(
                            out=dsums[:p, slot : slot + 1],
                            in_=h2[:p, 0 : w // 4],
                            axis=mybir.AxisListType.X,
                            op=op_add,
                        )

            # Alternating 15A/10D and 18A/7D pairs balances ACT vs DVE
            # busy time (measured best at ~155 us span)
            for pair in range(6):
                emit_tile_pair(2 * pair, 2 * pair + 1,
                               PLAN_A if pair % 2 == 0 else PLAN_B)
            emit_tile_pair(12, None, PLAN_B)

            # ---- tail: per-row loss and core-level sum ------------------
            den = small.tile([128, NRT], F32)
            nc.vector.tensor_reduce(
                out=den[:],
                in_=dsums[:].rearrange("p (t s) -> p t s", s=NSLOT),
                axis=mybir.AxisListType.X,
                op=op_add,
            )
            # scrub int32-saturation NaNs (saturated rows) to a huge finite
            den2 = small.tile([128, NRT], F32)
            nc.vector.tensor_scalar_min(out=den2[:], in0=den[:], scalar1=DEN_CAP)
            den_eps = small.tile([128, NRT], F32)
            nc.vector.scalar_tensor_tensor(
                out=den_eps[:], in0=den2[:], scalar=EPS, in1=e1_sb[:],
                op0=op_add, op1=op_add,
            )
            recip = small.tile([128, NRT], F32)
            nc.vector.reciprocal(out=recip[:], in_=den_eps[:])
            ratio = small.tile([128, NRT], F32)
            nc.vector.tensor_mul(ratio[:], e1_sb[:], recip[:])
            rateps = small.tile([128, NRT], F32)
            nc.vector.tensor_scalar_add(out=rateps[:], in0=ratio[:], scalar1=EPS)
            nc.vector.memset(rateps[64:128, NRT - 1 : NRT], 1.0)
            logv = small.tile([128, NRT], F32)
            lsum = small.tile([128, 1], F32)
            nc.scalar.activation(
                out=logv[:], in_=rateps[:], func=ln_f, accum_out=lsum[:]
            )
            tot_ps = psD.tile([128, BLK], F32, tag="psd")
            nc.tensor.matmul(
                out=tot_ps[0:1, 0:1], lhsT=lsum[:], rhs=ones_p[:],
                start=True, stop=True,
            )
            tot_sb = small.tile([1, 1], F32)
            nc.vector.tensor_copy(out=tot_sb[:], in_=tot_ps[0:1, 0:1])
            nc.sync.dma_start(out=out[:], in_=tot_sb[:])

    if not nc.is_finalized():
        nc.finalize()
    return nc


_NC_CACHE: list = []


def _get_nc() -> bass.Bass:
    if not _NC_CACHE:
        _NC_CACHE.append(_build())
    return _NC_CACHE[0]


_RUNNER_CACHE: list = []


def _get_runner():
    """Build the sharded PJRT executable once and reuse it across calls."""
    if _RUNNER_CACHE:
        return _RUNNER_CACHE[0]

    import jax
    import numpy as _np
    from jax.sharding import Mesh, PartitionSpec
    from jax.experimental.shard_map import shard_map
    from concourse import mybir as _mybir
    from concourse.bass2jax import (
        _bass_exec_p,
        install_neuronx_cc_hook,
        partition_id_tensor,
    )

    nc = _get_nc()
    install_neuronx_cc_hook()
    partition_name = nc.partition_id_tensor.name if nc.partition_id_tensor else None

    in_names, out_names, out_avals, zero_outs = [], [], [], []
    for alloc in nc.m.functions[0].allocations:
        if not isinstance(alloc, _mybir.MemoryLocationSet):
            continue
        name = alloc.memorylocations[0].name
        if alloc.kind == "ExternalInput":
            if name != partition_name:
                in_names.append(name)
        elif alloc.kind == "ExternalOutput":
            shape = tuple(alloc.tensor_shape)
            dtype = _mybir.dt.np(alloc.dtype)
            out_names.append(name)
            out_avals.append(jax.core.ShapedArray(shape, dtype))
            zero_outs.append(_np.zeros(shape, dtype))
    n_params = len(in_names)
    n_outs = len(out_avals)
    all_in_names = list(in_names) + list(out_names)
    if partition_name is not None:
        all_in_names.append(partition_name)

    def _body(*args):
        operands = list(args)
        if partition_name is not None:
            operands.append(partition_id_tensor())
        outs = _bass_exec_p.bind(
            *operands,
            out_avals=tuple(out_avals),
            in_names=tuple(all_in_names),
            out_names=tuple(out_names),
            lowering_input_output_aliases=(),
            sim_require_finite=False,
            sim_require_nnan=False,
            nc=nc,
        )
        return tuple(outs)

    devices = jax.devices()[:NCORES]
    mesh = Mesh(_np.asarray(devices), ("core",))
    spec_of = {
        "ae_tb": PartitionSpec("core"),
        "bias_in": PartitionSpec("core"),
        "e1_in": PartitionSpec("core"),
        "e_tb": PartitionSpec(),
    }
    in_specs = tuple(spec_of[nm] for nm in in_names) + (
        PartitionSpec("core"),
    ) * n_outs
    out_specs = (PartitionSpec("core"),) * n_outs
    donate = tuple(range(n_params, n_params + n_outs))
    sharded = jax.jit(
        shard_map(
            _body, mesh=mesh, in_specs=in_specs, out_specs=out_specs, check_rep=False
        ),
        donate_argnums=donate,
        keep_unused=True,
    )

    state = (sharded, in_names, out_names, out_avals, zero_outs)
    _RUNNER_CACHE.append(state)
    return state


def _to_bf16(x: np.ndarray):
    import ml_dtypes

    return x.astype(ml_dtypes.bfloat16)


def _prep_feeds(proj_main, proj_ema):
    """Full inputs -> (ae stacked, bias stacked, e1 stacked, e2 bf16)."""
    import ml_dtypes

    pm = np.ascontiguousarray(np.asarray(proj_main, dtype=np.float32))
    pe = np.ascontiguousarray(np.asarray(proj_ema, dtype=np.float32))
    at_full = np.ascontiguousarray(pm.transpose(1, 0, 2, 3).reshape(C, N))
    et_full = np.ascontiguousarray(pe.transpose(1, 0, 2, 3).reshape(C, N))

    pos = (at_full * et_full).sum(axis=0, dtype=np.float32)  # (N,) raw dots

    # per-row shift m' = pos + MARGIN (fp32); ACT bias = -10*m'
    bias_act = (-10.0 * (pos + np.float32(MARGIN))).astype(np.float32)
    e1 = np.exp(10.0 * pos.astype(np.float64)
                + bias_act.astype(np.float64)).astype(np.float32)

    a_scaled = _to_bf16(at_full * np.float32(ALPHA))     # (64, N) bf16
    e_b16 = _to_bf16(et_full)                            # (64, N) bf16
    ae_full = np.empty((128, N), dtype=ml_dtypes.bfloat16)
    ae_full[0:64] = a_scaled
    ae_full[64:128] = a_scaled
    e2 = np.empty((128, N), dtype=ml_dtypes.bfloat16)
    e2[0:64] = e_b16
    e2[64:128] = e_b16

    def layout_rows(v, pad=0.0):
        vp = np.full(NCORES * NRT * 128, pad, dtype=np.float32)
        for core in range(NCORES):
            vp[core * NRT * 128 : core * NRT * 128 + R] = v[
                core * R : (core + 1) * R
            ]
        return np.ascontiguousarray(
            vp.reshape(NCORES, NRT, 128).transpose(0, 2, 1).reshape(
                NCORES * 128, NRT
            )
        )

    bias_in = layout_rows(bias_act)
    e1_in = layout_rows(e1, pad=0.0)
    ae_sh = np.ascontiguousarray(
        np.asarray(ae_full).reshape(128, NCORES, R).transpose(1, 0, 2).reshape(
            NCORES * 128, R
        )
    )
    return ae_sh, bias_in, e1_in, np.ascontiguousarray(e2)


def _trace_in_maps(np_inputs):
    """Per-core input dicts for run_bass_kernel_spmd (trace harness)."""
    ae_sh, bias_in, e1_in, e2 = _prep_feeds(
        np_inputs["proj_main"], np_inputs["proj_ema"]
    )
    maps = []
    for core in range(NCORES):
        maps.append(
            {
                "ae_tb": np.ascontiguousarray(
                    ae_sh[core * 128 : (core + 1) * 128]
                ),
                "bias_in": np.ascontiguousarray(
                    bias_in[core * 128 : (core + 1) * 128]
                ),
                "e1_in": np.ascontiguousarray(
                    e1_in[core * 128 : (core + 1) * 128]
                ),
                "e_tb": e2,
            }
        )
    return maps


def kernel(proj_main, proj_ema, label_main, label_ema, patch_num):
    # labels / patch_num never influence the loss; only the projections do.
    ae_sh, bias_in, e1_in, e2 = _prep_feeds(proj_main, proj_ema)

    sharded, in_names, out_names, out_avals, zero_outs = _get_runner()
    feed = {"ae_tb": ae_sh, "bias_in": bias_in, "e1_in": e1_in, "e_tb": e2}
    args = [feed[nm] for nm in in_names]
    args += [
        np.zeros((NCORES * z.shape[0], *z.shape[1:]), z.dtype) for z in zero_outs
    ]
    out_arrs = sharded(*args)
    outs = np.asarray(out_arrs[out_names.index("out")]).reshape(NCORES)
    return np.float32(-float(outs.sum()) / N)


if __name__ == "__main__":
    _build()
    print("build OK")


# revision 50
# speedup vs baseline: 1.3121x; 1.0123x over previous
"""Trainium2 Bass kernel for nn_DenseContrastive (dense contrastive loss).

Math (per the fused reference):
    A = anchors (N, c), E = ema features (N, c), N = 12800, c = 64
    pos_i   = (A_i . E_i) / TEMP
    l_ij    = (A_i . E_j) / TEMP
    den_i   = sum_j exp(l_ij - s_i)            (s_i = per-row shift)
    r_i     = e1_i / (den_i + EPS + e1_i),  e1_i = exp(pos_i - s_i)
    loss_i  = -log(r_i + EPS);   out = mean_i loss_i

Shift choice (the structural win): s_i = pos_i + MARGIN, known on the
host -- no row-max pass.  The denominator always contains the diagonal
term exp(-MARGIN), so r <= ~1/2; if any exp overflows (row max > ~88
logits above s) then den saturates and loss = -log(EPS), which is
exactly what the reference computes for such a row (its softmax ratio
underflows below EPS).  Verified to rel err ~5e-7 vs the fp32 reference.

PE tiling (the throughput win): K = 64 contraction channels only fills
half the 128-row PE array, and a 64-partition moving stream runs at half
rate.  A and E are duplicated into both partition halves and row tiles
are processed in PAIRS: tile_position (0,0) computes row-tile t0 on PE
rows 0-63 while (64,0) computes t1 on rows 64-127, concurrently
(measured ~3x matmul throughput vs unpaired).

exp work is split across ACT and DVE per row tile (25 x 512 blocks,
15/10 and 18/7 on alternating pairs to balance engine busy time):
  ACT: table exp of scale*PSUM + bias_row (per-partition AP);
      accum_out gives the row sums for free.
  DVE: bf16 Schraudolph -- i16 = rne(max(PSUM + Brow, 0)), whose bit
      pattern read as bf16 is ~exp(10x) (ALPHA = 10*log2e*2^7 is folded
      into A; Brow = B16 - ALPHA*(pos+MARGIN), a per-partition scalar
      AP, folds the exponent bias and the row shift; the high side
      saturates the int16 convert to 0x7FFF = bf16 NaN, which a final
      min(den, 3e38) scrubs -- DVE min takes the non-NaN operand, and
      such rows are saturated anyway).  Row sums run as a packed-bf16
      tensor_tensor ADD tree (2 elem/cycle) plus a short 1x reduce.
e1 is computed on the host from the same shift, so numerator and
denominator shifts cancel exactly.

Sharding: N rows split across 8 cores (1600 each); E' (128 x 12800 bf16,
duplicated halves) replicated per core.  Core returns sum_i log(r_i+EPS);
host combines: loss = -sum/N.
"""

import sys

for _p in ("/opt/trn_rl_repo",):
    if _p not in sys.path:
        sys.path.insert(0, _p)

import numpy as np

import concourse.bass as bass
import concourse.bacc as bacc
import concourse.tile as tile
from concourse import mybir

TEMP = 0.1
EPS = 1e-8
B_, C, H, W = 2, 64, 80, 80
N = B_ * H * W           # 12800 anchors
NCORES = 8
R = N // NCORES          # 1600 rows per core
BLK = 512                # logit columns per PSUM bank
NBLK = N // BLK          # 25
MARGIN = 0.5             # raw-dot units; e1 ~ e^-5

LOG2E = 1.4426950408889634
# bf16-domain Schraudolph: exp(10*x) ~ bitcast16 of round(ALPHA*x + Brow)
ALPHA = np.float32(10.0 * LOG2E * (1 << 7))           # 1846.6497
SCHRAUD_C = 486408.0 / 65536.0                        # ~7.42
BCONST = np.float32(127.0 * (1 << 7) - SCHRAUD_C)     # ~16248.58
ACT_SCALE = np.float32(10.0 / float(ALPHA))
DEN_CAP = 3.0e38

F32 = mybir.dt.float32
I16 = mybir.dt.int16
BF16 = mybir.dt.bfloat16

# 1600 rows -> 6 pairs of 128-row tiles + one 64-row tile
ROW_TILES = [(i * 128, 128) for i in range(12)] + [(1536, 64)]
NRT = len(ROW_TILES)
NSLOT = 11               # up to 6 ACT accum slots + 5 reduce slots

# Alternating per-pair block plans balance ACT vs DVE at ~16.6/8.4:
# PLAN_A = 15 ACT / 10 DVE, PLAN_B = 18 ACT / 7 DVE.  (A uniform 17/8
# plan and a 2:4 mix both measured worse -- the alternating pair mix
# is the empirical optimum.)
PLAN_A = [
    ((0, 1, 2), (3, 4)),
    ((5, 6, 7), (8, 9)),
    ((10, 11, 12), (13, 14)),
    ((15, 16, 17), (18, 19)),
    ((20, 21, 22), (23, 24)),
]
PLAN_B = [
    ((0, 1, 2), (3, 4)),
    ((5, 6, 7), (8, 9)),
    ((10, 11, 12), (13, 14)),
    ((15, 16, 17), (18,)),
    ((19, 20, 21), ()),
    ((22, 23, 24), ()),
]


def _build() -> bass.Bass:
    nc = bacc.Bacc("TRN2", target_bir_lowering=False)
    ae_tb = nc.declare_dram_parameter("ae_tb", [128, R], BF16, isOutput=False)
    e_tb = nc.declare_dram_parameter("e_tb", [128, N], BF16, isOutput=False)
    bias_in = nc.declare_dram_parameter("bias_in", [128, NRT], F32, isOutput=False)
    e1_in = nc.declare_dram_parameter("e1_in", [128, NRT], F32, isOutput=False)
    out = nc.declare_dram_parameter("out", [1, 1], F32, isOutput=True)

    exp_f = mybir.ActivationFunctionType.Exp
    ln_f = mybir.ActivationFunctionType.Ln
    op_add = mybir.AluOpType.add
    op_max = mybir.AluOpType.max
    op_min = mybir.AluOpType.min
    op_mult = mybir.AluOpType.mult

    with tile.TileContext(nc) as tc:
        with (
            tc.tile_pool(name="big", bufs=1) as big,
            tc.tile_pool(name="small", bufs=1) as small,
            tc.tile_pool(name="trash", bufs=2) as trash,
            tc.tile_pool(name="intb", bufs=4) as intb,
            tc.tile_pool(name="tadd", bufs=2) as tadd,
            tc.tile_pool(name="psA", bufs=2, space="PSUM") as psA,
            tc.tile_pool(name="psD", bufs=2, space="PSUM") as psD,
        ):
            # ---- resident SBUF data -------------------------------------
            et_b = big.tile([128, N], BF16)     # E' dup halves (3.3 MB)
            aet_b = big.tile([128, R], BF16)    # A' shard, dup halves
            bias_sb = small.tile([128, NRT], F32)
            e1_sb = small.tile([128, NRT], F32)
            nc.sync.dma_start(out=aet_b[:], in_=ae_tb[:])
            nc.sync.dma_start(out=bias_sb[:], in_=bias_in[:])
            nc.sync.dma_start(out=e1_sb[:], in_=e1_in[:])
            for k in range(8):
                s = slice(k * (N // 8), (k + 1) * (N // 8))
                nc.sync.dma_start(out=et_b[:, s], in_=e_tb[:, s])

            ones_p = small.tile([128, 1], F32)
            nc.vector.memset(ones_p, 1.0)

            # Schraudolph per-row exponent bias: Brow = (ALPHA/10)*bias + B
            b_rows = small.tile([128, NRT], F32)
            nc.vector.tensor_scalar(
                out=b_rows[:], in0=bias_sb[:],
                scalar1=float(ALPHA) / 10.0, scalar2=float(BCONST),
                op0=op_mult, op1=op_add,
            )
            dsums = small.tile([128, NRT * NSLOT], F32)
            nc.vector.memset(dsums[:], 0.0)

            def emit_tile_pair(t0, t1, plan):
                """Row tiles t0 (PE rows 0-63) and t1 (rows 64-127), paired.
                t1 may be None (odd tail tile, unpaired)."""
                tiles = [(t0, 0)] + ([(t1, 64)] if t1 is not None else [])
                for g, (blocks, dblocks) in enumerate(plan):
                    psas, its = [], []
                    for (t, base) in tiles:
                        psas.append(
                            psA.tile([128, 3 * BLK], F32, tag="psa",
                                     name=f"psa_{t}_{g}")
                        )
                    for j, b in enumerate(blocks):
                        for k, (t, base) in enumerate(tiles):
                            r0, p = ROW_TILES[t]
                            kw = {} if t1 is None else {
                                "tile_position": (base, 0)}
                            nc.tensor.matmul(
                                out=psas[k][:p, j * BLK : (j + 1) * BLK],
                                lhsT=aet_b[base : base + 64, r0 : r0 + p],
                                rhs=et_b[base : base + 64,
                                         b * BLK : (b + 1) * BLK],
                                start=True, stop=True, **kw,
                            )
                    nb = len(blocks)
                    for k, (t, base) in enumerate(tiles):
                        r0, p = ROW_TILES[t]
                        scr = trash.tile([128, 3 * BLK], BF16, tag="scr")
                        nc.scalar.activation(
                            out=scr[:p, 0 : nb * BLK],
                            in_=psas[k][:p, 0 : nb * BLK],
                            func=exp_f,
                            scale=float(ACT_SCALE),
                            bias=bias_sb[:p, t : t + 1],
                            accum_out=dsums[:p, t * NSLOT + g : t * NSLOT + g + 1],
                        )
                    if not dblocks:
                        continue
                    nd = len(dblocks)
                    for (t, base) in tiles:
                        its.append(
                            intb.tile([128, 2 * BLK], I16, tag="intb",
                                      name=f"intb_{t}_{g}")
                        )
                    for h, b in enumerate(dblocks):
                        for k, (t, base) in enumerate(tiles):
                            r0, p = ROW_TILES[t]
                            kw = {} if t1 is None else {
                                "tile_position": (base, 0)}
                            psd = psD.tile([128, BLK], F32, tag="psd")
                            nc.tensor.matmul(
                                out=psd[:p, :],
                                lhsT=aet_b[base : base + 64, r0 : r0 + p],
                                rhs=et_b[base : base + 64,
                                         b * BLK : (b + 1) * BLK],
                                start=True, stop=True, **kw,
                            )
                            nc.vector.tensor_scalar(
                                out=its[k][:p, h * BLK : (h + 1) * BLK],
                                in0=psd[:p, :],
                                scalar1=b_rows[:p, t : t + 1],
                                scalar2=0.0,
                                op0=op_add, op1=op_max,
                            )
                    for k, (t, base) in enumerate(tiles):
                        r0, p = ROW_TILES[t]
                        slot = t * NSLOT + 6 + g
                        # fused halving-add + row sum in ONE native DVE op:
                        # out = (1.0*v_lo) + v_hi; slot = sum(out), fp32
                        # accumulation (scalar_tensor_tensor is native ISA,
                        # unlike the custom-DVE reduce ops which crash)
                        w = nd * BLK
                        v = its[k][:p, 0:w].bitcast(BF16)
                        h1 = tadd.tile([128, BLK], BF16, tag="h1")
                        nc.vector.scalar_tensor_tensor(
                            out=h1[:p, 0 : w // 2],
                            in0=v[:, 0 : w // 2],
                            scalar=1.0,
                            in1=v[:, w // 2 :],
                            op0=op_mult,
                            op1=op_add,
                            accum_out=dsums[:p, slot : slot + 1],
                        )

            # Alternating 15A/10D and 18A/7D pairs balances ACT vs DVE
            # busy time (measured best at ~155 us span)
            for pair in range(6):
                emit_tile_pair(2 * pair, 2 * pair + 1,
                               PLAN_A if pair % 2 == 0 else PLAN_B)
            emit_tile_pair(12, None, PLAN_B)

            # ---- tail: per-row loss and core-level sum ------------------
            den = small.tile([128, NRT], F32)
            nc.vector.tensor_reduce(
                out=den[:],
                in_=dsums[:].rearrange("p (t s) -> p t s", s=NSLOT),
                axis=mybir.AxisListType.X,
                op=op_add,
            )
            # scrub int32-saturation NaNs (saturated rows) to a huge finite
            den2 = small.tile([128, NRT], F32)
            nc.vector.tensor_scalar_min(out=den2[:], in0=den[:], scalar1=DEN_CAP)
            den_eps = small.tile([128, NRT], F32)
            nc.vector.scalar_tensor_tensor(
                out=den_eps[:], in0=den2[:], scalar=EPS, in1=e1_sb[:],
                op0=op_add, op1=op_add,
            )
            recip = small.tile([128, NRT], F32)
            nc.vector.reciprocal(out=recip[:], in_=den_eps[:])
            ratio = small.tile([128, NRT], F32)
            nc.vector.tensor_mul(ratio[:], e1_sb[:], recip[:])
            rateps = small.tile([128, NRT], F32)
            nc.vector.tensor_scalar_add(out=rateps[:], in0=ratio[:], scalar1=EPS)
            nc.vector.memset(rateps[64:128, NRT - 1 : NRT], 1.0)
            logv = small.tile([128, NRT], F32)
            lsum = small.tile([128, 1], F32)
            nc.scalar.activation(
                out=logv[:], in_=rateps[:], func=ln_f, accum_out=lsum[:]
            )
            tot_ps = psD.tile([128, BLK], F32, tag="psd")
            nc.tensor.matmul(
                out=tot_ps[0:1, 0:1], lhsT=lsum[:], rhs=ones_p[:],
                start=True, stop=True,
            )
            tot_sb = small.tile([1, 1], F32)
            nc.vector.tensor_copy(out=tot_sb[:], in_=tot_ps[0:1, 0:1])
            nc.sync.dma_start(out=out[:], in_=tot_sb[:])

    if not nc.is_finalized():
        nc.finalize()
    return nc


_NC_CACHE: list = []


def _get_nc() -> bass.Bass:
    if not _NC_CACHE:
        _NC_CACHE.append(_build())
    return _NC_CACHE[0]


_RUNNER_CACHE: list = []


def _get_runner():
    """Build the sharded PJRT executable once and reuse it across calls."""
    if _RUNNER_CACHE:
        return _RUNNER_CACHE[0]

    import jax
    import numpy as _np
    from jax.sharding import Mesh, PartitionSpec
    from jax.experimental.shard_map import shard_map
    from concourse import mybir as _mybir
    from concourse.bass2jax import (
        _bass_exec_p,
        install_neuronx_cc_hook,
        partition_id_tensor,
    )

    nc = _get_nc()
    install_neuronx_cc_hook()
    partition_name = nc.partition_id_tensor.name if nc.partition_id_tensor else None

    in_names, out_names, out_avals, zero_outs = [], [], [], []
    for alloc in nc.m.functions[0].allocations:
        if not isinstance(alloc, _mybir.MemoryLocationSet):
            continue
        name = alloc.memorylocations[0].name
        if alloc.kind == "ExternalInput":
            if name != partition_name:
                in_names.append(name)
        elif alloc.kind == "ExternalOutput":
            shape = tuple(alloc.tensor_shape)
            dtype = _mybir.dt.np(alloc.dtype)
            out_names.append(name)
            out_avals.append(jax.core.ShapedArray(shape, dtype))
            zero_outs.append(_np.zeros(shape, dtype))
    n_params = len(in_names)
    n_outs = len(out_avals)
    all_in_names = list(in_names) + list(out_names)
    if partition_name is not None:
        all_in_names.append(partition_name)

    def _body(*args):
        operands = list(args)
        if partition_name is not None:
            operands.append(partition_id_tensor())
        outs = _bass_exec_p.bind(
            *operands,
            out_avals=tuple(out_avals),
            in_names=tuple(all_in_names),
            out_names=tuple(out_names),
            lowering_input_output_aliases=(),
            sim_require_finite=False,
            sim_require_nnan=False,
            nc=nc,
        )
        return tuple(outs)

    devices = jax.devices()[:NCORES]
    mesh = Mesh(_np.asarray(devices), ("core",))
    spec_of = {
        "ae_tb": PartitionSpec("core"),
        "bias_in": PartitionSpec("core"),
        "e1_in": PartitionSpec("core"),
        "e_tb": PartitionSpec(),
    }
    in_specs = tuple(spec_of[nm] for nm in in_names) + (
        PartitionSpec("core"),
    ) * n_outs
    out_specs = (PartitionSpec("core"),) * n_outs
    donate = tuple(range(n_params, n_params + n_outs))
    sharded = jax.jit(
        shard_map(
            _body, mesh=mesh, in_specs=in_specs, out_specs=out_specs, check_rep=False
        ),
        donate_argnums=donate,
        keep_unused=True,
    )

    state = (sharded, in_names, out_names, out_avals, zero_outs)
    _RUNNER_CACHE.append(state)
    return state


def _to_bf16(x: np.ndarray):
    import ml_dtypes

    return x.astype(ml_dtypes.bfloat16)


def _prep_feeds(proj_main, proj_ema):
    """Full inputs -> (ae stacked, bias stacked, e1 stacked, e2 bf16)."""
    import ml_dtypes

    pm = np.ascontiguousarray(np.asarray(proj_main, dtype=np.float32))
    pe = np.ascontiguousarray(np.asarray(proj_ema, dtype=np.float32))
    at_full = np.ascontiguousarray(pm.transpose(1, 0, 2, 3).reshape(C, N))
    et_full = np.ascontiguousarray(pe.transpose(1, 0, 2, 3).reshape(C, N))

    pos = (at_full * et_full).sum(axis=0, dtype=np.float32)  # (N,) raw dots

    # per-row shift m' = pos + MARGIN (fp32); ACT bias = -10*m'
    bias_act = (-10.0 * (pos + np.float32(MARGIN))).astype(np.float32)
    e1 = np.exp(10.0 * pos.astype(np.float64)
                + bias_act.astype(np.float64)).astype(np.float32)

    a_scaled = _to_bf16(at_full * np.float32(ALPHA))     # (64, N) bf16
    e_b16 = _to_bf16(et_full)                            # (64, N) bf16
    ae_full = np.empty((128, N), dtype=ml_dtypes.bfloat16)
    ae_full[0:64] = a_scaled
    ae_full[64:128] = a_scaled
    e2 = np.empty((128, N), dtype=ml_dtypes.bfloat16)
    e2[0:64] = e_b16
    e2[64:128] = e_b16

    def layout_rows(v, pad=0.0):
        vp = np.full(NCORES * NRT * 128, pad, dtype=np.float32)
        for core in range(NCORES):
            vp[core * NRT * 128 : core * NRT * 128 + R] = v[
                core * R : (core + 1) * R
            ]
        return np.ascontiguousarray(
            vp.reshape(NCORES, NRT, 128).transpose(0, 2, 1).reshape(
                NCORES * 128, NRT
            )
        )

    bias_in = layout_rows(bias_act)
    e1_in = layout_rows(e1, pad=0.0)
    ae_sh = np.ascontiguousarray(
        np.asarray(ae_full).reshape(128, NCORES, R).transpose(1, 0, 2).reshape(
            NCORES * 128, R
        )
    )
    return ae_sh, bias_in, e1_in, np.ascontiguousarray(e2)


def _trace_in_maps(np_inputs):
    """Per-core input dicts for run_bass_kernel_spmd (trace harness)."""
    ae_sh, bias_in, e1_in, e2 = _prep_feeds(
        np_inputs["proj_main"], np_inputs["proj_ema"]
    )
    maps = []
    for core in range(NCORES):
        maps.append(
            {
                "ae_tb": np.ascontiguousarray(
                    ae_sh[core * 128 : (core + 1) * 128]
                ),
                "bias_in": np.ascontiguousarray(
                    bias_in[core * 128 : (core + 1) * 128]
                ),
                "e1_in": np.ascontiguousarray(
                    e1_in[core * 128 : (core + 1) * 128]
                ),
                "e_tb": e2,
            }
        )
    return maps


def kernel(proj_main, proj_ema, label_main, label_ema, patch_num):
    # labels / patch_num never influence the loss; only the projections do.
    ae_sh, bias_in, e1_in, e2 = _prep_feeds(proj_main, proj_ema)

    sharded, in_names, out_names, out_avals, zero_outs = _get_runner()
    feed = {"ae_tb": ae_sh, "bias_in": bias_in, "e1_in": e1_in, "e_tb": e2}
    args = [feed[nm] for nm in in_names]
    args += [
        np.zeros((NCORES * z.shape[0], *z.shape[1:]), z.dtype) for z in zero_outs
    ]
    out_arrs = sharded(*args)
    outs = np.asarray(out_arrs[out_names.index("out")]).reshape(NCORES)
    return np.float32(-float(outs.sum()) / N)


if __name__ == "__main__":
    _build()
    print("build OK")


# revision 51
# speedup vs baseline: 1.3460x; 1.0259x over previous
"""Trainium2 Bass kernel for nn_DenseContrastive (dense contrastive loss).

Math (per the fused reference):
    A = anchors (N, c), E = ema features (N, c), N = 12800, c = 64
    pos_i   = (A_i . E_i) / TEMP
    l_ij    = (A_i . E_j) / TEMP
    den_i   = sum_j exp(l_ij - s_i)            (s_i = per-row shift)
    r_i     = e1_i / (den_i + EPS + e1_i),  e1_i = exp(pos_i - s_i)
    loss_i  = -log(r_i + EPS);   out = mean_i loss_i

Shift choice (the structural win): s_i = pos_i + MARGIN, known on the
host -- no row-max pass.  The denominator always contains the diagonal
term exp(-MARGIN), so r <= ~1/2; if any exp overflows (row max > ~88
logits above s) then den saturates and loss = -log(EPS), which is
exactly what the reference computes for such a row (its softmax ratio
underflows below EPS).  Verified to rel err ~5e-7 vs the fp32 reference.

PE tiling (the throughput win): K = 64 contraction channels only fills
half the 128-row PE array, and a 64-partition moving stream runs at half
rate.  A and E are duplicated into both partition halves and row tiles
are processed in PAIRS: tile_position (0,0) computes row-tile t0 on PE
rows 0-63 while (64,0) computes t1 on rows 64-127, concurrently
(measured ~3x matmul throughput vs unpaired).

exp work is split across ACT and DVE per row tile (25 x 512 blocks,
15/10 and 18/7 on alternating pairs to balance engine busy time):
  ACT: table exp of scale*PSUM + bias_row (per-partition AP);
      accum_out gives the row sums for free.
  DVE: bf16 Schraudolph -- i16 = rne(max(PSUM + Brow, 0)), whose bit
      pattern read as bf16 is ~exp(10x) (ALPHA = 10*log2e*2^7 is folded
      into A; Brow = B16 - ALPHA*(pos+MARGIN), a per-partition scalar
      AP, folds the exponent bias and the row shift; the high side
      saturates the int16 convert to 0x7FFF = bf16 NaN, which a final
      min(den, 3e38) scrubs -- DVE min takes the non-NaN operand, and
      such rows are saturated anyway).  Row sums run as a packed-bf16
      tensor_tensor ADD tree (2 elem/cycle) plus a short 1x reduce.
e1 is computed on the host from the same shift, so numerator and
denominator shifts cancel exactly.

Sharding: N rows split across 8 cores (1600 each); E' (128 x 12800 bf16,
duplicated halves) replicated per core.  Core returns sum_i log(r_i+EPS);
host combines: loss = -sum/N.
"""

import sys

for _p in ("/opt/trn_rl_repo",):
    if _p not in sys.path:
        sys.path.insert(0, _p)

import numpy as np

import concourse.bass as bass
import concourse.bacc as bacc
import concourse.tile as tile
from concourse import mybir

TEMP = 0.1
EPS = 1e-8
B_, C, H, W = 2, 64, 80, 80
N = B_ * H * W           # 12800 anchors
NCORES = 8
R = N // NCORES          # 1600 rows per core
BLK = 512                # logit columns per PSUM bank
NBLK = N // BLK          # 25
MARGIN = 0.5             # raw-dot units; e1 ~ e^-5

LOG2E = 1.4426950408889634
# bf16-domain Schraudolph: exp(10*x) ~ bitcast16 of round(ALPHA*x + Brow)
ALPHA = np.float32(10.0 * LOG2E * (1 << 7))           # 1846.6497
SCHRAUD_C = 486408.0 / 65536.0                        # ~7.42
BCONST = np.float32(127.0 * (1 << 7) - SCHRAUD_C)     # ~16248.58
ACT_SCALE = np.float32(10.0 / float(ALPHA))
DEN_CAP = 3.0e38

F32 = mybir.dt.float32
I16 = mybir.dt.int16
BF16 = mybir.dt.bfloat16

# 1600 rows -> 6 pairs of 128-row tiles + one 64-row tile
ROW_TILES = [(i * 128, 128) for i in range(12)] + [(1536, 64)]
NRT = len(ROW_TILES)
NSLOT = 11               # up to 6 ACT accum slots + 5 reduce slots

# Alternating per-pair block plans balance ACT vs DVE at ~16.6/8.4:
# PLAN_A = 15 ACT / 10 DVE, PLAN_B = 18 ACT / 7 DVE.  (A uniform 17/8
# plan and a 2:4 mix both measured worse -- the alternating pair mix
# is the empirical optimum.)
PLAN_A = [
    ((0, 1, 2), (3, 4)),
    ((5, 6, 7), (8, 9)),
    ((10, 11, 12), (13, 14)),
    ((15, 16, 17), (18, 19)),
    ((20, 21, 22), (23, 24)),
]
PLAN_B = [
    ((0, 1, 2), (3, 4)),
    ((5, 6, 7), (8, 9)),
    ((10, 11, 12), (13, 14)),
    ((15, 16, 17), (18,)),
    ((19, 20, 21), ()),
    ((22, 23, 24), ()),
]


def _build() -> bass.Bass:
    nc = bacc.Bacc("TRN2", target_bir_lowering=False)
    ae_tb = nc.declare_dram_parameter("ae_tb", [128, R], BF16, isOutput=False)
    e_tb = nc.declare_dram_parameter("e_tb", [128, N], BF16, isOutput=False)
    bias_in = nc.declare_dram_parameter("bias_in", [128, NRT], F32, isOutput=False)
    e1_in = nc.declare_dram_parameter("e1_in", [128, NRT], F32, isOutput=False)
    out = nc.declare_dram_parameter("out", [1, 1], F32, isOutput=True)

    exp_f = mybir.ActivationFunctionType.Exp
    ln_f = mybir.ActivationFunctionType.Ln
    op_add = mybir.AluOpType.add
    op_max = mybir.AluOpType.max
    op_min = mybir.AluOpType.min
    op_mult = mybir.AluOpType.mult

    with tile.TileContext(nc) as tc:
        with (
            tc.tile_pool(name="big", bufs=1) as big,
            tc.tile_pool(name="small", bufs=1) as small,
            tc.tile_pool(name="trash", bufs=2) as trash,
            tc.tile_pool(name="intb", bufs=4) as intb,
            tc.tile_pool(name="tadd", bufs=2) as tadd,
            tc.tile_pool(name="psA", bufs=2, space="PSUM") as psA,
            tc.tile_pool(name="psD", bufs=2, space="PSUM") as psD,
        ):
            # ---- resident SBUF data -------------------------------------
            et_b = big.tile([128, N], BF16)     # E' dup halves (3.3 MB)
            aet_b = big.tile([128, R], BF16)    # A' shard, dup halves
            bias_sb = small.tile([128, NRT], F32)
            e1_sb = small.tile([128, NRT], F32)
            nc.sync.dma_start(out=aet_b[:], in_=ae_tb[:])
            nc.sync.dma_start(out=bias_sb[:], in_=bias_in[:])
            nc.sync.dma_start(out=e1_sb[:], in_=e1_in[:])
            for k in range(8):
                s = slice(k * (N // 8), (k + 1) * (N // 8))
                nc.sync.dma_start(out=et_b[:, s], in_=e_tb[:, s])

            ones_p = small.tile([128, 1], F32)
            nc.vector.memset(ones_p, 1.0)

            # Schraudolph per-row exponent bias: Brow = (ALPHA/10)*bias + B
            b_rows = small.tile([128, NRT], F32)
            nc.vector.tensor_scalar(
                out=b_rows[:], in0=bias_sb[:],
                scalar1=float(ALPHA) / 10.0, scalar2=float(BCONST),
                op0=op_mult, op1=op_add,
            )
            dsums = small.tile([128, NRT * NSLOT], F32)
            nc.vector.memset(dsums[:], 0.0)

            def emit_tile_pair(t0, t1, plan):
                """Row tiles t0 (PE rows 0-63) and t1 (rows 64-127), paired.
                t1 may be None (odd tail tile, unpaired)."""
                tiles = [(t0, 0)] + ([(t1, 64)] if t1 is not None else [])
                for g, (blocks, dblocks) in enumerate(plan):
                    psas, its = [], []
                    for (t, base) in tiles:
                        psas.append(
                            psA.tile([128, 3 * BLK], F32, tag="psa",
                                     name=f"psa_{t}_{g}")
                        )
                    for j, b in enumerate(blocks):
                        for k, (t, base) in enumerate(tiles):
                            r0, p = ROW_TILES[t]
                            kw = {} if t1 is None else {
                                "tile_position": (base, 0)}
                            nc.tensor.matmul(
                                out=psas[k][:p, j * BLK : (j + 1) * BLK],
                                lhsT=aet_b[base : base + 64, r0 : r0 + p],
                                rhs=et_b[base : base + 64,
                                         b * BLK : (b + 1) * BLK],
                                start=True, stop=True, **kw,
                            )
                    nb = len(blocks)
                    for k, (t, base) in enumerate(tiles):
                        r0, p = ROW_TILES[t]
                        scr = trash.tile([128, 3 * BLK], BF16, tag="scr")
                        nc.scalar.activation(
                            out=scr[:p, 0 : nb * BLK],
                            in_=psas[k][:p, 0 : nb * BLK],
                            func=exp_f,
                            scale=float(ACT_SCALE),
                            bias=bias_sb[:p, t : t + 1],
                            accum_out=dsums[:p, t * NSLOT + g : t * NSLOT + g + 1],
                        )
                    if not dblocks:
                        continue
                    nd = len(dblocks)
                    for (t, base) in tiles:
                        its.append(
                            intb.tile([128, 2 * BLK], I16, tag="intb",
                                      name=f"intb_{t}_{g}")
                        )
                    for h, b in enumerate(dblocks):
                        for k, (t, base) in enumerate(tiles):
                            r0, p = ROW_TILES[t]
                            kw = {} if t1 is None else {
                                "tile_position": (base, 0)}
                            psd = psD.tile([128, BLK], F32, tag="psd")
                            nc.tensor.matmul(
                                out=psd[:p, :],
                                lhsT=aet_b[base : base + 64, r0 : r0 + p],
                                rhs=et_b[base : base + 64,
                                         b * BLK : (b + 1) * BLK],
                                start=True, stop=True, **kw,
                            )
                            nc.vector.tensor_scalar(
                                out=its[k][:p, h * BLK : (h + 1) * BLK],
                                in0=psd[:p, :],
                                scalar1=b_rows[:p, t : t + 1],
                                scalar2=0.0,
                                op0=op_add, op1=op_max,
                            )
                    for k, (t, base) in enumerate(tiles):
                        r0, p = ROW_TILES[t]
                        slot = t * NSLOT + 6 + g
                        # fused halving-add + row sum in ONE native DVE op:
                        # out = (1.0*v_lo) + v_hi; slot = sum(out), fp32
                        # accumulation (scalar_tensor_tensor is native ISA,
                        # unlike the custom-DVE reduce ops which crash)
                        w = nd * BLK
                        v = its[k][:p, 0:w].bitcast(BF16)
                        h1 = tadd.tile([128, BLK], BF16, tag="h1")
                        nc.vector.scalar_tensor_tensor(
                            out=h1[:p, 0 : w // 2],
                            in0=v[:, 0 : w // 2],
                            scalar=1.0,
                            in1=v[:, w // 2 :],
                            op0=op_mult,
                            op1=op_add,
                            accum_out=dsums[:p, slot : slot + 1],
                        )

            # 4 pairs at 15A/10D + 2 at 18A/7D balances ACT vs DVE busy
            # time now that the fused STT reduce lightened the DVE
            for pair in range(6):
                emit_tile_pair(2 * pair, 2 * pair + 1,
                               PLAN_B if pair in (1, 3) else PLAN_A)
            emit_tile_pair(12, None, PLAN_B)

            # ---- tail: per-row loss and core-level sum ------------------
            den = small.tile([128, NRT], F32)
            nc.vector.tensor_reduce(
                out=den[:],
                in_=dsums[:].rearrange("p (t s) -> p t s", s=NSLOT),
                axis=mybir.AxisListType.X,
                op=op_add,
            )
            # scrub int32-saturation NaNs (saturated rows) to a huge finite
            den2 = small.tile([128, NRT], F32)
            nc.vector.tensor_scalar_min(out=den2[:], in0=den[:], scalar1=DEN_CAP)
            den_eps = small.tile([128, NRT], F32)
            nc.vector.scalar_tensor_tensor(
                out=den_eps[:], in0=den2[:], scalar=EPS, in1=e1_sb[:],
                op0=op_add, op1=op_add,
            )
            recip = small.tile([128, NRT], F32)
            nc.vector.reciprocal(out=recip[:], in_=den_eps[:])
            ratio = small.tile([128, NRT], F32)
            nc.vector.tensor_mul(ratio[:], e1_sb[:], recip[:])
            rateps = small.tile([128, NRT], F32)
            nc.vector.tensor_scalar_add(out=rateps[:], in0=ratio[:], scalar1=EPS)
            nc.vector.memset(rateps[64:128, NRT - 1 : NRT], 1.0)
            logv = small.tile([128, NRT], F32)
            lsum = small.tile([128, 1], F32)
            nc.scalar.activation(
                out=logv[:], in_=rateps[:], func=ln_f, accum_out=lsum[:]
            )
            tot_ps = psD.tile([128, BLK], F32, tag="psd")
            nc.tensor.matmul(
                out=tot_ps[0:1, 0:1], lhsT=lsum[:], rhs=ones_p[:],
                start=True, stop=True,
            )
            tot_sb = small.tile([1, 1], F32)
            nc.vector.tensor_copy(out=tot_sb[:], in_=tot_ps[0:1, 0:1])
            nc.sync.dma_start(out=out[:], in_=tot_sb[:])

    if not nc.is_finalized():
        nc.finalize()
    return nc


_NC_CACHE: list = []


def _get_nc() -> bass.Bass:
    if not _NC_CACHE:
        _NC_CACHE.append(_build())
    return _NC_CACHE[0]


_RUNNER_CACHE: list = []


def _get_runner():
    """Build the sharded PJRT executable once and reuse it across calls."""
    if _RUNNER_CACHE:
        return _RUNNER_CACHE[0]

    import jax
    import numpy as _np
    from jax.sharding import Mesh, PartitionSpec
    from jax.experimental.shard_map import shard_map
    from concourse import mybir as _mybir
    from concourse.bass2jax import (
        _bass_exec_p,
        install_neuronx_cc_hook,
        partition_id_tensor,
    )

    nc = _get_nc()
    install_neuronx_cc_hook()
    partition_name = nc.partition_id_tensor.name if nc.partition_id_tensor else None

    in_names, out_names, out_avals, zero_outs = [], [], [], []
    for alloc in nc.m.functions[0].allocations:
        if not isinstance(alloc, _mybir.MemoryLocationSet):
            continue
        name = alloc.memorylocations[0].name
        if alloc.kind == "ExternalInput":
            if name != partition_name:
                in_names.append(name)
        elif alloc.kind == "ExternalOutput":
            shape = tuple(alloc.tensor_shape)
            dtype = _mybir.dt.np(alloc.dtype)
            out_names.append(name)
            out_avals.append(jax.core.ShapedArray(shape, dtype))
            zero_outs.append(_np.zeros(shape, dtype))
    n_params = len(in_names)
    n_outs = len(out_avals)
    all_in_names = list(in_names) + list(out_names)
    if partition_name is not None:
        all_in_names.append(partition_name)

    def _body(*args):
        operands = list(args)
        if partition_name is not None:
            operands.append(partition_id_tensor())
        outs = _bass_exec_p.bind(
            *operands,
            out_avals=tuple(out_avals),
            in_names=tuple(all_in_names),
            out_names=tuple(out_names),
            lowering_input_output_aliases=(),
            sim_require_finite=False,
            sim_require_nnan=False,
            nc=nc,
        )
        return tuple(outs)

    devices = jax.devices()[:NCORES]
    mesh = Mesh(_np.asarray(devices), ("core",))
    spec_of = {
        "ae_tb": PartitionSpec("core"),
        "bias_in": PartitionSpec("core"),
        "e1_in": PartitionSpec("core"),
        "e_tb": PartitionSpec(),
    }
    in_specs = tuple(spec_of[nm] for nm in in_names) + (
        PartitionSpec("core"),
    ) * n_outs
    out_specs = (PartitionSpec("core"),) * n_outs
    donate = tuple(range(n_params, n_params + n_outs))
    sharded = jax.jit(
        shard_map(
            _body, mesh=mesh, in_specs=in_specs, out_specs=out_specs, check_rep=False
        ),
        donate_argnums=donate,
        keep_unused=True,
    )

    state = (sharded, in_names, out_names, out_avals, zero_outs)
    _RUNNER_CACHE.append(state)
    return state


def _to_bf16(x: np.ndarray):
    import ml_dtypes

    return x.astype(ml_dtypes.bfloat16)


def _prep_feeds(proj_main, proj_ema):
    """Full inputs -> (ae stacked, bias stacked, e1 stacked, e2 bf16)."""
    import ml_dtypes

    pm = np.ascontiguousarray(np.asarray(proj_main, dtype=np.float32))
    pe = np.ascontiguousarray(np.asarray(proj_ema, dtype=np.float32))
    at_full = np.ascontiguousarray(pm.transpose(1, 0, 2, 3).reshape(C, N))
    et_full = np.ascontiguousarray(pe.transpose(1, 0, 2, 3).reshape(C, N))

    pos = (at_full * et_full).sum(axis=0, dtype=np.float32)  # (N,) raw dots

    # per-row shift m' = pos + MARGIN (fp32); ACT bias = -10*m'
    bias_act = (-10.0 * (pos + np.float32(MARGIN))).astype(np.float32)
    e1 = np.exp(10.0 * pos.astype(np.float64)
                + bias_act.astype(np.float64)).astype(np.float32)

    a_scaled = _to_bf16(at_full * np.float32(ALPHA))     # (64, N) bf16
    e_b16 = _to_bf16(et_full)                            # (64, N) bf16
    ae_full = np.empty((128, N), dtype=ml_dtypes.bfloat16)
    ae_full[0:64] = a_scaled
    ae_full[64:128] = a_scaled
    e2 = np.empty((128, N), dtype=ml_dtypes.bfloat16)
    e2[0:64] = e_b16
    e2[64:128] = e_b16

    def layout_rows(v, pad=0.0):
        vp = np.full(NCORES * NRT * 128, pad, dtype=np.float32)
        for core in range(NCORES):
            vp[core * NRT * 128 : core * NRT * 128 + R] = v[
                core * R : (core + 1) * R
            ]
        return np.ascontiguousarray(
            vp.reshape(NCORES, NRT, 128).transpose(0, 2, 1).reshape(
                NCORES * 128, NRT
            )
        )

    bias_in = layout_rows(bias_act)
    e1_in = layout_rows(e1, pad=0.0)
    ae_sh = np.ascontiguousarray(
        np.asarray(ae_full).reshape(128, NCORES, R).transpose(1, 0, 2).reshape(
            NCORES * 128, R
        )
    )
    return ae_sh, bias_in, e1_in, np.ascontiguousarray(e2)


def _trace_in_maps(np_inputs):
    """Per-core input dicts for run_bass_kernel_spmd (trace harness)."""
    ae_sh, bias_in, e1_in, e2 = _prep_feeds(
        np_inputs["proj_main"], np_inputs["proj_ema"]
    )
    maps = []
    for core in range(NCORES):
        maps.append(
            {
                "ae_tb": np.ascontiguousarray(
                    ae_sh[core * 128 : (core + 1) * 128]
                ),
                "bias_in": np.ascontiguousarray(
                    bias_in[core * 128 : (core + 1) * 128]
                ),
                "e1_in": np.ascontiguousarray(
                    e1_in[core * 128 : (core + 1) * 128]
                ),
                "e_tb": e2,
            }
        )
    return maps


def kernel(proj_main, proj_ema, label_main, label_ema, patch_num):
    # labels / patch_num never influence the loss; only the projections do.
    ae_sh, bias_in, e1_in, e2 = _prep_feeds(proj_main, proj_ema)

    sharded, in_names, out_names, out_avals, zero_outs = _get_runner()
    feed = {"ae_tb": ae_sh, "bias_in": bias_in, "e1_in": e1_in, "e_tb": e2}
    args = [feed[nm] for nm in in_names]
    args += [
        np.zeros((NCORES * z.shape[0], *z.shape[1:]), z.dtype) for z in zero_outs
    ]
    out_arrs = sharded(*args)
    outs = np.asarray(out_arrs[out_names.index("out")]).reshape(NCORES)
    return np.float32(-float(outs.sum()) / N)


if __name__ == "__main__":
    _build()
    print("build OK")


# revision 54
# speedup vs baseline: 1.3463x; 1.0002x over previous
"""Trainium2 Bass kernel for nn_DenseContrastive (dense contrastive loss).

Math (per the fused reference):
    A = anchors (N, c), E = ema features (N, c), N = 12800, c = 64
    pos_i   = (A_i . E_i) / TEMP
    l_ij    = (A_i . E_j) / TEMP
    den_i   = sum_j exp(l_ij - s_i)            (s_i = per-row shift)
    r_i     = e1_i / (den_i + EPS + e1_i),  e1_i = exp(pos_i - s_i)
    loss_i  = -log(r_i + EPS);   out = mean_i loss_i

Shift choice (the structural win): s_i = pos_i + MARGIN, known on the
host -- no row-max pass.  The denominator always contains the diagonal
term exp(-MARGIN), so r <= ~1/2; if any exp overflows (row max > ~88
logits above s) then den saturates and loss = -log(EPS), which is
exactly what the reference computes for such a row (its softmax ratio
underflows below EPS).  Verified to rel err ~5e-7 vs the fp32 reference.

PE tiling (the throughput win): K = 64 contraction channels only fills
half the 128-row PE array, and a 64-partition moving stream runs at half
rate.  A and E are duplicated into both partition halves and row tiles
are processed in PAIRS: tile_position (0,0) computes row-tile t0 on PE
rows 0-63 while (64,0) computes t1 on rows 64-127, concurrently
(measured ~3x matmul throughput vs unpaired).

exp work is split across ACT and DVE per row tile (25 x 512 blocks;
15/10 on four pairs and 18/7 on two, balancing engine busy time):
  ACT: table exp of scale*PSUM + bias_row (per-partition AP);
      accum_out gives the row sums for free.
  DVE: bf16 Schraudolph -- i16 = rne(max(PSUM + Brow, 0)), whose bit
      pattern read as bf16 is ~exp(10x) (ALPHA = 10*log2e*2^7 is folded
      into A; Brow = B16 - ALPHA*(pos+MARGIN), a per-partition scalar
      AP, folds the exponent bias and the row shift; the high side
      saturates the int16 convert to 0x7FFF = bf16 NaN, which a final
      min(den, 3e38) scrubs -- DVE min takes the non-NaN operand, and
      such rows are saturated anyway).  Row sums fuse into one native
      scalar_tensor_tensor per unit: out = v_lo + v_hi with
      accum_out = sum(out) in fp32.
e1 is computed on the host from the same shift, so numerator and
denominator shifts cancel exactly.

Sharding: N rows split across 8 cores (1600 each); E' (128 x 12800 bf16,
duplicated halves) replicated per core.  Core returns sum_i log(r_i+EPS);
host combines: loss = -sum/N.
"""

import sys

for _p in ("/opt/trn_rl_repo",):
    if _p not in sys.path:
        sys.path.insert(0, _p)

import numpy as np

import concourse.bass as bass
import concourse.bacc as bacc
import concourse.tile as tile
from concourse import mybir

TEMP = 0.1
EPS = 1e-8
B_, C, H, W = 2, 64, 80, 80
N = B_ * H * W           # 12800 anchors
NCORES = 8
R = N // NCORES          # 1600 rows per core
BLK = 512                # logit columns per PSUM bank
NBLK = N // BLK          # 25
MARGIN = 0.5             # raw-dot units; e1 ~ e^-5

LOG2E = 1.4426950408889634
# bf16-domain Schraudolph: exp(10*x) ~ bitcast16 of round(ALPHA*x + Brow)
ALPHA = np.float32(10.0 * LOG2E * (1 << 7))           # 1846.6497
SCHRAUD_C = 486408.0 / 65536.0                        # ~7.42
BCONST = np.float32(127.0 * (1 << 7) - SCHRAUD_C)     # ~16248.58
ACT_SCALE = np.float32(10.0 / float(ALPHA))
DEN_CAP = 3.0e38

F32 = mybir.dt.float32
I16 = mybir.dt.int16
BF16 = mybir.dt.bfloat16

# 1600 rows -> 6 pairs of 128-row tiles + one 64-row tile
ROW_TILES = [(i * 128, 128) for i in range(12)] + [(1536, 64)]
NRT = len(ROW_TILES)
NSLOT = 11               # up to 6 ACT accum slots + 5 reduce slots

# Alternating per-pair block plans balance ACT vs DVE at ~16.6/8.4:
# PLAN_A = 15 ACT / 10 DVE, PLAN_B = 18 ACT / 7 DVE.  (A uniform 17/8
# plan and a 2:4 mix both measured worse -- the alternating pair mix
# is the empirical optimum.)
PLAN_A = [
    ((0, 1, 2), (3, 4)),
    ((5, 6, 7), (8, 9)),
    ((10, 11, 12), (13, 14)),
    ((15, 16, 17), (18, 19)),
    ((20, 21, 22), (23, 24)),
]
PLAN_B = [
    ((0, 1, 2), (3, 4)),
    ((5, 6, 7), (8, 9)),
    ((10, 11, 12), (13, 14)),
    ((15, 16, 17), (18,)),
    ((19, 20, 21), ()),
    ((22, 23, 24), ()),
]


def _build() -> bass.Bass:
    nc = bacc.Bacc("TRN2", target_bir_lowering=False)
    ae_tb = nc.declare_dram_parameter("ae_tb", [128, R], BF16, isOutput=False)
    e_tb = nc.declare_dram_parameter("e_tb", [128, N], BF16, isOutput=False)
    bias_in = nc.declare_dram_parameter("bias_in", [128, NRT], F32, isOutput=False)
    e1_in = nc.declare_dram_parameter("e1_in", [128, NRT], F32, isOutput=False)
    out = nc.declare_dram_parameter("out", [1, 1], F32, isOutput=True)

    exp_f = mybir.ActivationFunctionType.Exp
    ln_f = mybir.ActivationFunctionType.Ln
    op_add = mybir.AluOpType.add
    op_max = mybir.AluOpType.max
    op_min = mybir.AluOpType.min
    op_mult = mybir.AluOpType.mult

    with tile.TileContext(nc) as tc:
        with (
            tc.tile_pool(name="big", bufs=1) as big,
            tc.tile_pool(name="small", bufs=1) as small,
            tc.tile_pool(name="trash", bufs=2) as trash,
            tc.tile_pool(name="intb", bufs=4) as intb,
            tc.tile_pool(name="tadd", bufs=2) as tadd,
            tc.tile_pool(name="psA", bufs=2, space="PSUM") as psA,
            tc.tile_pool(name="psD", bufs=2, space="PSUM") as psD,
        ):
            # ---- resident SBUF data -------------------------------------
            et_b = big.tile([128, N], BF16)     # E' dup halves (3.3 MB)
            aet_b = big.tile([128, R], BF16)    # A' shard, dup halves
            bias_sb = small.tile([128, NRT], F32)
            e1_sb = small.tile([128, NRT], F32)
            nc.sync.dma_start(out=aet_b[:], in_=ae_tb[:])
            nc.sync.dma_start(out=bias_sb[:], in_=bias_in[:])
            nc.sync.dma_start(out=e1_sb[:], in_=e1_in[:])
            for k in range(8):
                s = slice(k * (N // 8), (k + 1) * (N // 8))
                nc.sync.dma_start(out=et_b[:, s], in_=e_tb[:, s])

            ones_p = small.tile([128, 1], F32)
            nc.vector.memset(ones_p, 1.0)

            # Schraudolph per-row exponent bias: Brow = (ALPHA/10)*bias + B
            b_rows = small.tile([128, NRT], F32)
            nc.vector.tensor_scalar(
                out=b_rows[:], in0=bias_sb[:],
                scalar1=float(ALPHA) / 10.0, scalar2=float(BCONST),
                op0=op_mult, op1=op_add,
            )
            dsums = small.tile([128, NRT * NSLOT], F32)
            nc.vector.memset(dsums[:], 0.0)

            def emit_tile_pair(t0, t1, plan):
                """Row tiles t0 (PE rows 0-63) and t1 (rows 64-127), paired.
                t1 may be None (odd tail tile, unpaired)."""
                tiles = [(t0, 0)] + ([(t1, 64)] if t1 is not None else [])
                for g, (blocks, dblocks) in enumerate(plan):
                    psas, its = [], []
                    for (t, base) in tiles:
                        psas.append(
                            psA.tile([128, 3 * BLK], F32, tag="psa",
                                     name=f"psa_{t}_{g}")
                        )
                    for j, b in enumerate(blocks):
                        for k, (t, base) in enumerate(tiles):
                            r0, p = ROW_TILES[t]
                            kw = {} if t1 is None else {
                                "tile_position": (base, 0)}
                            nc.tensor.matmul(
                                out=psas[k][:p, j * BLK : (j + 1) * BLK],
                                lhsT=aet_b[base : base + 64, r0 : r0 + p],
                                rhs=et_b[base : base + 64,
                                         b * BLK : (b + 1) * BLK],
                                start=True, stop=True, **kw,
                            )
                    nb = len(blocks)
                    for k, (t, base) in enumerate(tiles):
                        r0, p = ROW_TILES[t]
                        # exp output written in place to PSUM (faster ACT
                        # port than SBUF; values are trash -- the row sums
                        # ride accum_out)
                        nc.scalar.activation(
                            out=psas[k][:p, 0 : nb * BLK],
                            in_=psas[k][:p, 0 : nb * BLK],
                            func=exp_f,
                            scale=float(ACT_SCALE),
                            bias=bias_sb[:p, t : t + 1],
                            accum_out=dsums[:p, t * NSLOT + g : t * NSLOT + g + 1],
                        )
                    if not dblocks:
                        continue
                    nd = len(dblocks)
                    for (t, base) in tiles:
                        its.append(
                            intb.tile([128, 2 * BLK], I16, tag="intb",
                                      name=f"intb_{t}_{g}")
                        )
                    for h, b in enumerate(dblocks):
                        for k, (t, base) in enumerate(tiles):
                            r0, p = ROW_TILES[t]
                            kw = {} if t1 is None else {
                                "tile_position": (base, 0)}
                            psd = psD.tile([128, BLK], F32, tag="psd")
                            nc.tensor.matmul(
                                out=psd[:p, :],
                                lhsT=aet_b[base : base + 64, r0 : r0 + p],
                                rhs=et_b[base : base + 64,
                                         b * BLK : (b + 1) * BLK],
                                start=True, stop=True, **kw,
                            )
                            nc.vector.tensor_scalar(
                                out=its[k][:p, h * BLK : (h + 1) * BLK],
                                in0=psd[:p, :],
                                scalar1=b_rows[:p, t : t + 1],
                                scalar2=0.0,
                                op0=op_add, op1=op_max,
                            )
                    for k, (t, base) in enumerate(tiles):
                        r0, p = ROW_TILES[t]
                        slot = t * NSLOT + 6 + g
                        # fused halving-add + row sum in ONE native DVE op:
                        # out = (1.0*v_lo) + v_hi; slot = sum(out), fp32
                        # accumulation (scalar_tensor_tensor is native ISA,
                        # unlike the custom-DVE reduce ops which crash)
                        w = nd * BLK
                        v = its[k][:p, 0:w].bitcast(BF16)
                        h1 = tadd.tile([128, BLK], BF16, tag="h1")
                        nc.vector.scalar_tensor_tensor(
                            out=h1[:p, 0 : w // 2],
                            in0=v[:, 0 : w // 2],
                            scalar=1.0,
                            in1=v[:, w // 2 :],
                            op0=op_mult,
                            op1=op_add,
                            accum_out=dsums[:p, slot : slot + 1],
                        )

            # 4 pairs at 15A/10D + 2 at 18A/7D balances ACT vs DVE busy
            # time now that the fused STT reduce lightened the DVE
            for pair in range(6):
                emit_tile_pair(2 * pair, 2 * pair + 1,
                               PLAN_B if pair in (1, 3) else PLAN_A)
            emit_tile_pair(12, None, PLAN_B)

            # ---- tail: per-row loss and core-level sum ------------------
            den = small.tile([128, NRT], F32)
            nc.vector.tensor_reduce(
                out=den[:],
                in_=dsums[:].rearrange("p (t s) -> p t s", s=NSLOT),
                axis=mybir.AxisListType.X,
                op=op_add,
            )
            # scrub int32-saturation NaNs (saturated rows) to a huge finite
            den2 = small.tile([128, NRT], F32)
            nc.vector.tensor_scalar_min(out=den2[:], in0=den[:], scalar1=DEN_CAP)
            den_eps = small.tile([128, NRT], F32)
            nc.vector.scalar_tensor_tensor(
                out=den_eps[:], in0=den2[:], scalar=EPS, in1=e1_sb[:],
                op0=op_add, op1=op_add,
            )
            recip = small.tile([128, NRT], F32)
            nc.vector.reciprocal(out=recip[:], in_=den_eps[:])
            ratio = small.tile([128, NRT], F32)
            nc.vector.tensor_mul(ratio[:], e1_sb[:], recip[:])
            rateps = small.tile([128, NRT], F32)
            nc.vector.tensor_scalar_add(out=rateps[:], in0=ratio[:], scalar1=EPS)
            nc.vector.memset(rateps[64:128, NRT - 1 : NRT], 1.0)
            logv = small.tile([128, NRT], F32)
            lsum = small.tile([128, 1], F32)
            nc.scalar.activation(
                out=logv[:], in_=rateps[:], func=ln_f, accum_out=lsum[:]
            )
            tot_ps = psD.tile([128, BLK], F32, tag="psd")
            nc.tensor.matmul(
                out=tot_ps[0:1, 0:1], lhsT=lsum[:], rhs=ones_p[:],
                start=True, stop=True,
            )
            tot_sb = small.tile([1, 1], F32)
            nc.vector.tensor_copy(out=tot_sb[:], in_=tot_ps[0:1, 0:1])
            nc.sync.dma_start(out=out[:], in_=tot_sb[:])

    if not nc.is_finalized():
        nc.finalize()
    return nc


_NC_CACHE: list = []


def _get_nc() -> bass.Bass:
    if not _NC_CACHE:
        _NC_CACHE.append(_build())
    return _NC_CACHE[0]


_RUNNER_CACHE: list = []


def _get_runner():
    """Build the sharded PJRT executable once and reuse it across calls."""
    if _RUNNER_CACHE:
        return _RUNNER_CACHE[0]

    import jax
    import numpy as _np
    from jax.sharding import Mesh, PartitionSpec
    from jax.experimental.shard_map import shard_map
    from concourse import mybir as _mybir
    from concourse.bass2jax import (
        _bass_exec_p,
        install_neuronx_cc_hook,
        partition_id_tensor,
    )

    nc = _get_nc()
    install_neuronx_cc_hook()
    partition_name = nc.partition_id_tensor.name if nc.partition_id_tensor else None

    in_names, out_names, out_avals, zero_outs = [], [], [], []
    for alloc in nc.m.functions[0].allocations:
        if not isinstance(alloc, _mybir.MemoryLocationSet):
            continue
        name = alloc.memorylocations[0].name
        if alloc.kind == "ExternalInput":
            if name != partition_name:
                in_names.append(name)
        elif alloc.kind == "ExternalOutput":
            shape = tuple(alloc.tensor_shape)
            dtype = _mybir.dt.np(alloc.dtype)
            out_names.append(name)
            out_avals.append(jax.core.ShapedArray(shape, dtype))
            zero_outs.append(_np.zeros(shape, dtype))
    n_params = len(in_names)
    n_outs = len(out_avals)
    all_in_names = list(in_names) + list(out_names)
    if partition_name is not None:
        all_in_names.append(partition_name)

    def _body(*args):
        operands = list(args)
        if partition_name is not None:
            operands.append(partition_id_tensor())
        outs = _bass_exec_p.bind(
            *operands,
            out_avals=tuple(out_avals),
            in_names=tuple(all_in_names),
            out_names=tuple(out_names),
            lowering_input_output_aliases=(),
            sim_require_finite=False,
            sim_require_nnan=False,
            nc=nc,
        )
        return tuple(outs)

    devices = jax.devices()[:NCORES]
    mesh = Mesh(_np.asarray(devices), ("core",))
    spec_of = {
        "ae_tb": PartitionSpec("core"),
        "bias_in": PartitionSpec("core"),
        "e1_in": PartitionSpec("core"),
        "e_tb": PartitionSpec(),
    }
    in_specs = tuple(spec_of[nm] for nm in in_names) + (
        PartitionSpec("core"),
    ) * n_outs
    out_specs = (PartitionSpec("core"),) * n_outs
    donate = tuple(range(n_params, n_params + n_outs))
    sharded = jax.jit(
        shard_map(
            _body, mesh=mesh, in_specs=in_specs, out_specs=out_specs, check_rep=False
        ),
        donate_argnums=donate,
        keep_unused=True,
    )

    state = (sharded, in_names, out_names, out_avals, zero_outs)
    _RUNNER_CACHE.append(state)
    return state


def _to_bf16(x: np.ndarray):
    import ml_dtypes

    return x.astype(ml_dtypes.bfloat16)


def _prep_feeds(proj_main, proj_ema):
    """Full inputs -> (ae stacked, bias stacked, e1 stacked, e2 bf16)."""
    import ml_dtypes

    pm = np.ascontiguousarray(np.asarray(proj_main, dtype=np.float32))
    pe = np.ascontiguousarray(np.asarray(proj_ema, dtype=np.float32))
    at_full = np.ascontiguousarray(pm.transpose(1, 0, 2, 3).reshape(C, N))
    et_full = np.ascontiguousarray(pe.transpose(1, 0, 2, 3).reshape(C, N))

    pos = (at_full * et_full).sum(axis=0, dtype=np.float32)  # (N,) raw dots

    # per-row shift m' = pos + MARGIN (fp32); ACT bias = -10*m'
    bias_act = (-10.0 * (pos + np.float32(MARGIN))).astype(np.float32)
    e1 = np.exp(10.0 * pos.astype(np.float64)
                + bias_act.astype(np.float64)).astype(np.float32)

    a_scaled = _to_bf16(at_full * np.float32(ALPHA))     # (64, N) bf16
    e_b16 = _to_bf16(et_full)                            # (64, N) bf16
    ae_full = np.empty((128, N), dtype=ml_dtypes.bfloat16)
    ae_full[0:64] = a_scaled
    ae_full[64:128] = a_scaled
    e2 = np.empty((128, N), dtype=ml_dtypes.bfloat16)
    e2[0:64] = e_b16
    e2[64:128] = e_b16

    def layout_rows(v, pad=0.0):
        vp = np.full(NCORES * NRT * 128, pad, dtype=np.float32)
        for core in range(NCORES):
            vp[core * NRT * 128 : core * NRT * 128 + R] = v[
                core * R : (core + 1) * R
            ]
        return np.ascontiguousarray(
            vp.reshape(NCORES, NRT, 128).transpose(0, 2, 1).reshape(
                NCORES * 128, NRT
            )
        )

    bias_in = layout_rows(bias_act)
    e1_in = layout_rows(e1, pad=0.0)
    ae_sh = np.ascontiguousarray(
        np.asarray(ae_full).reshape(128, NCORES, R).transpose(1, 0, 2).reshape(
            NCORES * 128, R
        )
    )
    return ae_sh, bias_in, e1_in, np.ascontiguousarray(e2)


def _trace_in_maps(np_inputs):
    """Per-core input dicts for run_bass_kernel_spmd (trace harness)."""
    ae_sh, bias_in, e1_in, e2 = _prep_feeds(
        np_inputs["proj_main"], np_inputs["proj_ema"]
    )
    maps = []
    for core in range(NCORES):
        maps.append(
            {
                "ae_tb": np.ascontiguousarray(
                    ae_sh[core * 128 : (core + 1) * 128]
                ),
                "bias_in": np.ascontiguousarray(
                    bias_in[core * 128 : (core + 1) * 128]
                ),
                "e1_in": np.ascontiguousarray(
                    e1_in[core * 128 : (core + 1) * 128]
                ),
                "e_tb": e2,
            }
        )
    return maps


def kernel(proj_main, proj_ema, label_main, label_ema, patch_num):
    # labels / patch_num never influence the loss; only the projections do.
    ae_sh, bias_in, e1_in, e2 = _prep_feeds(proj_main, proj_ema)

    sharded, in_names, out_names, out_avals, zero_outs = _get_runner()
    feed = {"ae_tb": ae_sh, "bias_in": bias_in, "e1_in": e1_in, "e_tb": e2}
    args = [feed[nm] for nm in in_names]
    args += [
        np.zeros((NCORES * z.shape[0], *z.shape[1:]), z.dtype) for z in zero_outs
    ]
    out_arrs = sharded(*args)
    outs = np.asarray(out_arrs[out_names.index("out")]).reshape(NCORES)
    return np.float32(-float(outs.sum()) / N)


if __name__ == "__main__":
    _build()
    print("build OK")


# revision 55
# speedup vs baseline: 1.3480x; 1.0013x over previous
"""Trainium2 Bass kernel for nn_DenseContrastive (dense contrastive loss).

Math (per the fused reference):
    A = anchors (N, c), E = ema features (N, c), N = 12800, c = 64
    pos_i   = (A_i . E_i) / TEMP
    l_ij    = (A_i . E_j) / TEMP
    den_i   = sum_j exp(l_ij - s_i)            (s_i = per-row shift)
    r_i     = e1_i / (den_i + EPS + e1_i),  e1_i = exp(pos_i - s_i)
    loss_i  = -log(r_i + EPS);   out = mean_i loss_i

Shift choice (the structural win): s_i = pos_i + MARGIN, known on the
host -- no row-max pass.  The denominator always contains the diagonal
term exp(-MARGIN), so r <= ~1/2; if any exp overflows (row max > ~88
logits above s) then den saturates and loss = -log(EPS), which is
exactly what the reference computes for such a row (its softmax ratio
underflows below EPS).  Verified to rel err ~5e-7 vs the fp32 reference.

PE tiling (the throughput win): K = 64 contraction channels only fills
half the 128-row PE array, and a 64-partition moving stream runs at half
rate.  A and E are duplicated into both partition halves and row tiles
are processed in PAIRS: tile_position (0,0) computes row-tile t0 on PE
rows 0-63 while (64,0) computes t1 on rows 64-127, concurrently
(measured ~3x matmul throughput vs unpaired).

exp work is split across ACT and DVE per row tile (25 x 512 blocks;
15/10 on four pairs and 18/7 on two, balancing engine busy time):
  ACT: table exp of scale*PSUM + bias_row (per-partition AP);
      accum_out gives the row sums for free.
  DVE: bf16 Schraudolph -- i16 = rne(max(PSUM + Brow, 0)), whose bit
      pattern read as bf16 is ~exp(10x) (ALPHA = 10*log2e*2^7 is folded
      into A; Brow = B16 - ALPHA*(pos+MARGIN), a per-partition scalar
      AP, folds the exponent bias and the row shift; the high side
      saturates the int16 convert to 0x7FFF = bf16 NaN, which a final
      min(den, 3e38) scrubs -- DVE min takes the non-NaN operand, and
      such rows are saturated anyway).  Row sums fuse into one native
      scalar_tensor_tensor per unit: out = v_lo + v_hi with
      accum_out = sum(out) in fp32.
e1 is computed on the host from the same shift, so numerator and
denominator shifts cancel exactly.

Sharding: N rows split across 8 cores (1600 each); E' (128 x 12800 bf16,
duplicated halves) replicated per core.  Core returns sum_i log(r_i+EPS);
host combines: loss = -sum/N.
"""

import sys

for _p in ("/opt/trn_rl_repo",):
    if _p not in sys.path:
        sys.path.insert(0, _p)

import numpy as np

import concourse.bass as bass
import concourse.bacc as bacc
import concourse.tile as tile
from concourse import mybir

TEMP = 0.1
EPS = 1e-8
B_, C, H, W = 2, 64, 80, 80
N = B_ * H * W           # 12800 anchors
NCORES = 8
R = N // NCORES          # 1600 rows per core
BLK = 512                # logit columns per PSUM bank
NBLK = N // BLK          # 25
MARGIN = 0.5             # raw-dot units; e1 ~ e^-5

LOG2E = 1.4426950408889634
# bf16-domain Schraudolph: exp(10*x) ~ bitcast16 of round(ALPHA*x + Brow)
ALPHA = np.float32(10.0 * LOG2E * (1 << 7))           # 1846.6497
SCHRAUD_C = 486408.0 / 65536.0                        # ~7.42
BCONST = np.float32(127.0 * (1 << 7) - SCHRAUD_C)     # ~16248.58
ACT_SCALE = np.float32(10.0 / float(ALPHA))
DEN_CAP = 3.0e38

F32 = mybir.dt.float32
I16 = mybir.dt.int16
BF16 = mybir.dt.bfloat16

# 1600 rows -> 6 pairs of 128-row tiles + one 64-row tile
ROW_TILES = [(i * 128, 128) for i in range(12)] + [(1536, 64)]
NRT = len(ROW_TILES)
NSLOT = 11               # up to 6 ACT accum slots + 5 reduce slots

# Alternating per-pair block plans balance ACT vs DVE at ~16.6/8.4:
# PLAN_A = 15 ACT / 10 DVE, PLAN_B = 18 ACT / 7 DVE.  (A uniform 17/8
# plan and a 2:4 mix both measured worse -- the alternating pair mix
# is the empirical optimum.)
PLAN_A = [
    ((0, 1, 2), (3, 4)),
    ((5, 6, 7), (8, 9)),
    ((10, 11, 12), (13, 14)),
    ((15, 16, 17), (18, 19)),
    ((20, 21, 22), (23, 24)),
]
PLAN_B = [
    ((0, 1, 2), (3, 4)),
    ((5, 6, 7), (8, 9)),
    ((10, 11, 12), (13, 14)),
    ((15, 16, 17), (18,)),
    ((19, 20, 21), ()),
    ((22, 23, 24), ()),
]


def _build() -> bass.Bass:
    nc = bacc.Bacc("TRN2", target_bir_lowering=False)
    ae_tb = nc.declare_dram_parameter("ae_tb", [128, R], BF16, isOutput=False)
    e_tb = nc.declare_dram_parameter("e_tb", [128, N], BF16, isOutput=False)
    bias_in = nc.declare_dram_parameter("bias_in", [128, NRT], F32, isOutput=False)
    e1_in = nc.declare_dram_parameter("e1_in", [128, NRT], F32, isOutput=False)
    out = nc.declare_dram_parameter("out", [1, 1], F32, isOutput=True)

    exp_f = mybir.ActivationFunctionType.Exp
    ln_f = mybir.ActivationFunctionType.Ln
    op_add = mybir.AluOpType.add
    op_max = mybir.AluOpType.max
    op_min = mybir.AluOpType.min
    op_mult = mybir.AluOpType.mult

    with tile.TileContext(nc) as tc:
        with (
            tc.tile_pool(name="big", bufs=1) as big,
            tc.tile_pool(name="small", bufs=1) as small,
            tc.tile_pool(name="trash", bufs=2) as trash,
            tc.tile_pool(name="intb", bufs=4) as intb,
            tc.tile_pool(name="tadd", bufs=2) as tadd,
            tc.tile_pool(name="psA", bufs=2, space="PSUM") as psA,
            tc.tile_pool(name="psD", bufs=2, space="PSUM") as psD,
        ):
            # ---- resident SBUF data -------------------------------------
            et_b = big.tile([128, N], BF16)     # E' dup halves (3.3 MB)
            aet_b = big.tile([128, R], BF16)    # A' shard, dup halves
            bias_sb = small.tile([128, NRT], F32)
            e1_sb = small.tile([128, NRT], F32)
            nc.sync.dma_start(out=aet_b[:], in_=ae_tb[:])
            nc.sync.dma_start(out=bias_sb[:], in_=bias_in[:])
            nc.sync.dma_start(out=e1_sb[:], in_=e1_in[:])
            for k in range(8):
                s = slice(k * (N // 8), (k + 1) * (N // 8))
                nc.sync.dma_start(out=et_b[:, s], in_=e_tb[:, s])

            ones_p = small.tile([128, 1], F32)
            nc.vector.memset(ones_p, 1.0)

            # Schraudolph per-row exponent bias: Brow = (ALPHA/10)*bias + B
            b_rows = small.tile([128, NRT], F32)
            nc.vector.tensor_scalar(
                out=b_rows[:], in0=bias_sb[:],
                scalar1=float(ALPHA) / 10.0, scalar2=float(BCONST),
                op0=op_mult, op1=op_add,
            )
            dsums = small.tile([128, NRT * NSLOT], F32)
            nc.vector.memset(dsums[:], 0.0)

            def emit_tile_pair(t0, t1, plan):
                """Row tiles t0 (PE rows 0-63) and t1 (rows 64-127), paired.
                t1 may be None (odd tail tile, unpaired)."""
                tiles = [(t0, 0)] + ([(t1, 64)] if t1 is not None else [])
                for g, (blocks, dblocks) in enumerate(plan):
                    psas, its = [], []
                    for (t, base) in tiles:
                        psas.append(
                            psA.tile([128, 3 * BLK], F32, tag="psa",
                                     name=f"psa_{t}_{g}")
                        )
                    for j, b in enumerate(blocks):
                        for k, (t, base) in enumerate(tiles):
                            r0, p = ROW_TILES[t]
                            kw = {} if t1 is None else {
                                "tile_position": (base, 0)}
                            nc.tensor.matmul(
                                out=psas[k][:p, j * BLK : (j + 1) * BLK],
                                lhsT=aet_b[base : base + 64, r0 : r0 + p],
                                rhs=et_b[base : base + 64,
                                         b * BLK : (b + 1) * BLK],
                                start=True, stop=True, **kw,
                            )
                    nb = len(blocks)
                    for k, (t, base) in enumerate(tiles):
                        r0, p = ROW_TILES[t]
                        # exp output written in place to PSUM (faster ACT
                        # port than SBUF; values are trash -- the row sums
                        # ride accum_out)
                        nc.scalar.activation(
                            out=psas[k][:p, 0 : nb * BLK],
                            in_=psas[k][:p, 0 : nb * BLK],
                            func=exp_f,
                            scale=float(ACT_SCALE),
                            bias=bias_sb[:p, t : t + 1],
                            accum_out=dsums[:p, t * NSLOT + g : t * NSLOT + g + 1],
                        )
                    if not dblocks:
                        continue
                    nd = len(dblocks)
                    for (t, base) in tiles:
                        its.append(
                            intb.tile([128, 2 * BLK], I16, tag="intb",
                                      name=f"intb_{t}_{g}")
                        )
                    for h, b in enumerate(dblocks):
                        for k, (t, base) in enumerate(tiles):
                            r0, p = ROW_TILES[t]
                            kw = {} if t1 is None else {
                                "tile_position": (base, 0)}
                            psd = psD.tile([128, BLK], F32, tag="psd")
                            nc.tensor.matmul(
                                out=psd[:p, :],
                                lhsT=aet_b[base : base + 64, r0 : r0 + p],
                                rhs=et_b[base : base + 64,
                                         b * BLK : (b + 1) * BLK],
                                start=True, stop=True, **kw,
                            )
                            nc.vector.tensor_scalar(
                                out=its[k][:p, h * BLK : (h + 1) * BLK],
                                in0=psd[:p, :],
                                scalar1=b_rows[:p, t : t + 1],
                                scalar2=0.0,
                                op0=op_add, op1=op_max,
                            )
                    for k, (t, base) in enumerate(tiles):
                        r0, p = ROW_TILES[t]
                        slot = t * NSLOT + 6 + g
                        # fused halving-add + row sum in ONE native DVE op:
                        # out = (1.0*v_lo) + v_hi; slot = sum(out), fp32
                        # accumulation (scalar_tensor_tensor is native ISA,
                        # unlike the custom-DVE reduce ops which crash)
                        w = nd * BLK
                        v = its[k][:p, 0:w].bitcast(BF16)
                        h1 = tadd.tile([128, BLK], BF16, tag="h1")
                        nc.vector.scalar_tensor_tensor(
                            out=h1[:p, 0 : w // 2],
                            in0=v[:, 0 : w // 2],
                            scalar=1.0,
                            in1=v[:, w // 2 :],
                            op0=op_mult,
                            op1=op_add,
                            accum_out=dsums[:p, slot : slot + 1],
                        )

            # 4 pairs at 15A/10D + 2 at 18A/7D balances ACT vs DVE busy
            # time now that the fused STT reduce lightened the DVE
            # last pair runs the D-light plan so the kernel does not end
            # on the longest DVE drain chain
            for pair in range(6):
                emit_tile_pair(2 * pair, 2 * pair + 1,
                               PLAN_B if pair in (1, 5) else PLAN_A)
            emit_tile_pair(12, None, PLAN_B)

            # ---- tail: per-row loss and core-level sum ------------------
            den = small.tile([128, NRT], F32)
            nc.vector.tensor_reduce(
                out=den[:],
                in_=dsums[:].rearrange("p (t s) -> p t s", s=NSLOT),
                axis=mybir.AxisListType.X,
                op=op_add,
            )
            # scrub int32-saturation NaNs (saturated rows) to a huge finite
            den2 = small.tile([128, NRT], F32)
            nc.vector.tensor_scalar_min(out=den2[:], in0=den[:], scalar1=DEN_CAP)
            den_eps = small.tile([128, NRT], F32)
            nc.vector.scalar_tensor_tensor(
                out=den_eps[:], in0=den2[:], scalar=EPS, in1=e1_sb[:],
                op0=op_add, op1=op_add,
            )
            recip = small.tile([128, NRT], F32)
            nc.vector.reciprocal(out=recip[:], in_=den_eps[:])
            ratio = small.tile([128, NRT], F32)
            nc.vector.tensor_mul(ratio[:], e1_sb[:], recip[:])
            rateps = small.tile([128, NRT], F32)
            nc.vector.tensor_scalar_add(out=rateps[:], in0=ratio[:], scalar1=EPS)
            nc.vector.memset(rateps[64:128, NRT - 1 : NRT], 1.0)
            logv = small.tile([128, NRT], F32)
            lsum = small.tile([128, 1], F32)
            nc.scalar.activation(
                out=logv[:], in_=rateps[:], func=ln_f, accum_out=lsum[:]
            )
            tot_ps = psD.tile([128, BLK], F32, tag="psd")
            nc.tensor.matmul(
                out=tot_ps[0:1, 0:1], lhsT=lsum[:], rhs=ones_p[:],
                start=True, stop=True,
            )
            tot_sb = small.tile([1, 1], F32)
            nc.vector.tensor_copy(out=tot_sb[:], in_=tot_ps[0:1, 0:1])
            nc.sync.dma_start(out=out[:], in_=tot_sb[:])

    if not nc.is_finalized():
        nc.finalize()
    return nc


_NC_CACHE: list = []


def _get_nc() -> bass.Bass:
    if not _NC_CACHE:
        _NC_CACHE.append(_build())
    return _NC_CACHE[0]


_RUNNER_CACHE: list = []


def _get_runner():
    """Build the sharded PJRT executable once and reuse it across calls."""
    if _RUNNER_CACHE:
        return _RUNNER_CACHE[0]

    import jax
    import numpy as _np
    from jax.sharding import Mesh, PartitionSpec
    from jax.experimental.shard_map import shard_map
    from concourse import mybir as _mybir
    from concourse.bass2jax import (
        _bass_exec_p,
        install_neuronx_cc_hook,
        partition_id_tensor,
    )

    nc = _get_nc()
    install_neuronx_cc_hook()
    partition_name = nc.partition_id_tensor.name if nc.partition_id_tensor else None

    in_names, out_names, out_avals, zero_outs = [], [], [], []
    for alloc in nc.m.functions[0].allocations:
        if not isinstance(alloc, _mybir.MemoryLocationSet):
            continue
        name = alloc.memorylocations[0].name
        if alloc.kind == "ExternalInput":
            if name != partition_name:
                in_names.append(name)
        elif alloc.kind == "ExternalOutput":
            shape = tuple(alloc.tensor_shape)
            dtype = _mybir.dt.np(alloc.dtype)
            out_names.append(name)
            out_avals.append(jax.core.ShapedArray(shape, dtype))
            zero_outs.append(_np.zeros(shape, dtype))
    n_params = len(in_names)
    n_outs = len(out_avals)
    all_in_names = list(in_names) + list(out_names)
    if partition_name is not None:
        all_in_names.append(partition_name)

    def _body(*args):
        operands = list(args)
        if partition_name is not None:
            operands.append(partition_id_tensor())
        outs = _bass_exec_p.bind(
            *operands,
            out_avals=tuple(out_avals),
            in_names=tuple(all_in_names),
            out_names=tuple(out_names),
            lowering_input_output_aliases=(),
            sim_require_finite=False,
            sim_require_nnan=False,
            nc=nc,
        )
        return tuple(outs)

    devices = jax.devices()[:NCORES]
    mesh = Mesh(_np.asarray(devices), ("core",))
    spec_of = {
        "ae_tb": PartitionSpec("core"),
        "bias_in": PartitionSpec("core"),
        "e1_in": PartitionSpec("core"),
        "e_tb": PartitionSpec(),
    }
    in_specs = tuple(spec_of[nm] for nm in in_names) + (
        PartitionSpec("core"),
    ) * n_outs
    out_specs = (PartitionSpec("core"),) * n_outs
    donate = tuple(range(n_params, n_params + n_outs))
    sharded = jax.jit(
        shard_map(
            _body, mesh=mesh, in_specs=in_specs, out_specs=out_specs, check_rep=False
        ),
        donate_argnums=donate,
        keep_unused=True,
    )

    state = (sharded, in_names, out_names, out_avals, zero_outs)
    _RUNNER_CACHE.append(state)
    return state


def _to_bf16(x: np.ndarray):
    import ml_dtypes

    return x.astype(ml_dtypes.bfloat16)


def _prep_feeds(proj_main, proj_ema):
    """Full inputs -> (ae stacked, bias stacked, e1 stacked, e2 bf16)."""
    import ml_dtypes

    pm = np.ascontiguousarray(np.asarray(proj_main, dtype=np.float32))
    pe = np.ascontiguousarray(np.asarray(proj_ema, dtype=np.float32))
    at_full = np.ascontiguousarray(pm.transpose(1, 0, 2, 3).reshape(C, N))
    et_full = np.ascontiguousarray(pe.transpose(1, 0, 2, 3).reshape(C, N))

    pos = (at_full * et_full).sum(axis=0, dtype=np.float32)  # (N,) raw dots

    # per-row shift m' = pos + MARGIN (fp32); ACT bias = -10*m'
    bias_act = (-10.0 * (pos + np.float32(MARGIN))).astype(np.float32)
    e1 = np.exp(10.0 * pos.astype(np.float64)
                + bias_act.astype(np.float64)).astype(np.float32)

    a_scaled = _to_bf16(at_full * np.float32(ALPHA))     # (64, N) bf16
    e_b16 = _to_bf16(et_full)                            # (64, N) bf16
    ae_full = np.empty((128, N), dtype=ml_dtypes.bfloat16)
    ae_full[0:64] = a_scaled
    ae_full[64:128] = a_scaled
    e2 = np.empty((128, N), dtype=ml_dtypes.bfloat16)
    e2[0:64] = e_b16
    e2[64:128] = e_b16

    def layout_rows(v, pad=0.0):
        vp = np.full(NCORES * NRT * 128, pad, dtype=np.float32)
        for core in range(NCORES):
            vp[core * NRT * 128 : core * NRT * 128 + R] = v[
                core * R : (core + 1) * R
            ]
        return np.ascontiguousarray(
            vp.reshape(NCORES, NRT, 128).transpose(0, 2, 1).reshape(
                NCORES * 128, NRT
            )
        )

    bias_in = layout_rows(bias_act)
    e1_in = layout_rows(e1, pad=0.0)
    ae_sh = np.ascontiguousarray(
        np.asarray(ae_full).reshape(128, NCORES, R).transpose(1, 0, 2).reshape(
            NCORES * 128, R
        )
    )
    return ae_sh, bias_in, e1_in, np.ascontiguousarray(e2)


def _trace_in_maps(np_inputs):
    """Per-core input dicts for run_bass_kernel_spmd (trace harness)."""
    ae_sh, bias_in, e1_in, e2 = _prep_feeds(
        np_inputs["proj_main"], np_inputs["proj_ema"]
    )
    maps = []
    for core in range(NCORES):
        maps.append(
            {
                "ae_tb": np.ascontiguousarray(
                    ae_sh[core * 128 : (core + 1) * 128]
                ),
                "bias_in": np.ascontiguousarray(
                    bias_in[core * 128 : (core + 1) * 128]
                ),
                "e1_in": np.ascontiguousarray(
                    e1_in[core * 128 : (core + 1) * 128]
                ),
                "e_tb": e2,
            }
        )
    return maps


def kernel(proj_main, proj_ema, label_main, label_ema, patch_num):
    # labels / patch_num never influence the loss; only the projections do.
    ae_sh, bias_in, e1_in, e2 = _prep_feeds(proj_main, proj_ema)

    sharded, in_names, out_names, out_avals, zero_outs = _get_runner()
    feed = {"ae_tb": ae_sh, "bias_in": bias_in, "e1_in": e1_in, "e_tb": e2}
    args = [feed[nm] for nm in in_names]
    args += [
        np.zeros((NCORES * z.shape[0], *z.shape[1:]), z.dtype) for z in zero_outs
    ]
    out_arrs = sharded(*args)
    outs = np.asarray(out_arrs[out_names.index("out")]).reshape(NCORES)
    return np.float32(-float(outs.sum()) / N)


if __name__ == "__main__":
    _build()
    print("build OK")
